# revision 27
# baseline (speedup 1.0000x reference)
"""Trainium2 Bass kernel for nn_BlockConv (PointNet-style GNN block), 8 cores.

Algebraic core: msg_e = concat(x_src, pos_src-pos_dst) @ W + b
  = A[src] - C[dst], with A = concat(x,pos)@W (per-node table) and
  C = pos@W[-3:] - b (per-dst, constant within a segment).
  segment_max over dst = (gather+max of A rows) - C[dst]. Memory-bound.

Distribution: dst-sharded; per-core edge gathers from a replicated fp16
A-table in HBM via 4-queue SWDGE dma_gather (descriptor-rate limited:
~3ns/row on 4 queues vs ~9 on one). Tables, stages, h are fp16. Host
pre-transposes x/pos so table builds are straight fp16 matmuls; the
table uses a p-major row layout (row = base + p*G + g for batch column
p + 128*g) so each table write is one contiguous-per-partition DMA
instead of a 2048-descriptor strided store. Dst slots are degree-sorted
per side (lo/hi row windows for int16 idx) so gather pass k covers a
slot prefix. Dead (degree-0) slots get an exact v=0 via a 5th posm row
carrying -BIG through the C-matmul. The AllGather ships pre-BN v1^T in
two column chunks (first issued mid-merge) concurrently with the stats
AllReduce; conv2's build applies BN+relu on the fly (DVE) and folds
pos@W2p into h via host-computed P = W2p @ W2a^-1. Output is
feature-major in lo2-slot order; the host unpermutes.
"""
import sys
import numpy as np

if "/opt/trn_rl_repo" not in sys.path:
    sys.path.insert(0, "/opt/trn_rl_repo")

BIG_NEG = -60000.0
EPS = 1e-5
CB1 = 4096          # conv1 table build batch (columns)
CB2 = 2048          # conv2 table build batch (columns)

FULL_CFG = dict(N=50000, E=800000, CIN=64, COUT=128, NC=8,
                LO_LIM=32768, R=50432)
MINI_CFG = dict(N=2048, E=16384, CIN=64, COUT=128, NC=8,
                LO_LIM=1024, R=2432)
MID_CFG = dict(N=16384, E=262144, CIN=64, COUT=128, NC=8,
               LO_LIM=8192, R=16768)


def _ceil(a, b):
    return (a + b - 1) // b


def _wrap16(ids):
    """flat int list (len % 128 == 0) -> [128, len//16] int16 wrapped:
    unwrapped[j] = g[j%16, j//16], replicated over the 8 core groups."""
    a = np.asarray(ids, np.int64)
    assert a.size % 128 == 0 and a.min() >= 0 and a.max() < 32768
    g = a.reshape(a.size // 16, 16).T.astype(np.int16)   # [16, L/16]
    return np.tile(g, (8, 1))                            # [128, L/16]


def _row1(n, N):
    """p-major conv1 table row for node n (batches of CB1 columns)."""
    b = n // CB1
    o = n - b * CB1
    G = np.minimum(CB1 // 128, (N - b * CB1 + 127) // 128)
    return 1 + b * CB1 + (o % 128) * G + o // 128


def _row2(ct, slot, NSLOT):
    """p-major conv2 table row for (core ct, lo1-slot)."""
    c0 = (slot // CB2) * CB2
    w = np.minimum(CB2, NSLOT - c0)
    o = slot - c0
    return 1 + ct * NSLOT + c0 + (o % 128) * (w // 128) + o // 128


def _side_prep(rows, d_loc, NSLOT):
    """Degree-sort dst slots for one source window. rows = side-local table
    row ids per edge; d_loc = local dst node position per edge."""
    deg = np.bincount(d_loc, minlength=NSLOT)
    order = np.argsort(-deg, kind="stable")        # slot s -> node position
    slot_of = np.empty(NSLOT, np.int64)            # node position -> slot
    slot_of[order] = np.arange(NSLOT)
    isort = np.argsort(d_loc, kind="stable")
    starts = np.zeros(NSLOT + 1, np.int64)
    np.cumsum(deg, out=starts[1:])
    return {"deg": deg, "order": order, "slot_of": slot_of,
            "s_sorted": rows[isort], "starts": starts,
            "cnts": np.sort(deg)[::-1]}


def _build_sched(sides_by_core):
    """sched[si] = list of S_k (128-slot chunks) per pass, max across cores."""
    sched = []
    for si in range(2):
        Sk = []
        kmax = max(int(sd[si]["cnts"][0]) for sd in sides_by_core)
        for k in range(kmax):
            cnt = max(int((sd[si]["cnts"] > k).sum()) for sd in sides_by_core)
            if cnt == 0:
                break
            Sk.append(_ceil(cnt, 128))
        sched.append(Sk)
    return sched


def _build_gi(side, sched_side, dummy):
    blocks = []
    for k, S in enumerate(sched_side):
        L = S * 128
        ids = np.full(L, dummy, np.int64)
        nsl = int((side["cnts"] > k).sum())
        nodes = side["order"][:nsl]
        ids[:nsl] = side["s_sorted"][side["starts"][nodes] + k]
        blocks.append(_wrap16(ids))
    if not blocks:
        return np.zeros((128, 8), np.int16)
    return np.concatenate(blocks, axis=1)


def _posm_aug(pos, node, dead):
    """[5, NSLOT] fp16: rows 0-2 pos, row 3 = -1 (bias), row 4 = -BIG flag.
    Dead slots get rows 0-3 zeroed and row4=-BIG so C' = -BIG exactly and
    v = agg - C' = 0 exactly."""
    m = np.empty((5, node.size), np.float16)
    m[:3] = pos[node].T
    m[3] = -1.0
    m[4] = 0.0
    m[0:4, dead] = 0.0
    m[4, dead] = BIG_NEG
    return np.ascontiguousarray(m)


def host_prep(edge_index, pos, x, cfg):
    N, NC, LO_LIM, R = cfg["N"], cfg["NC"], cfg["LO_LIM"], cfg["R"]
    NLOC = N // NC
    SLOC = _ceil(NLOC, 128)
    NSLOT = SLOC * 128
    HID = R - 1 - LO_LIM          # hi-local dummy row (global row R-1)
    src = np.asarray(edge_index[0], np.int64)
    dst = np.asarray(edge_index[1], np.int64)
    core_of = dst // NLOC
    pos = np.asarray(pos, np.float32)
    x = np.asarray(x, np.float32)

    # ---- conv1: per-core lo/hi sides in p-major row space ----
    r1 = _row1(src, N)
    data1, slot1_lo_glob = [], np.empty(N, np.int64)
    for c in range(NC):
        m = core_of == c
        r = r1[m]
        d = dst[m] - c * NLOC
        sides = []
        for si, sel in ((0, r < LO_LIM), (1, r >= LO_LIM)):
            sides.append(_side_prep(r[sel] - (LO_LIM if si else 0),
                                    d[sel], NSLOT))
        data1.append(sides)
        slot1_lo_glob[c * NLOC:(c + 1) * NLOC] = sides[0]["slot_of"][:NLOC]
    sched1 = _build_sched(data1)

    # ---- conv2: rows in p-major (core, lo1-slot) space ----
    r2 = _row2(src // NLOC, slot1_lo_glob[src], NSLOT)
    data2 = []
    for c in range(NC):
        m = core_of == c
        r = r2[m]
        d = dst[m] - c * NLOC
        sides = []
        for si, sel in ((0, r < LO_LIM), (1, r >= LO_LIM)):
            sides.append(_side_prep(r[sel] - (LO_LIM if si else 0),
                                    d[sel], NSLOT))
        data2.append(sides)
    sched2 = _build_sched(data2)

    # ---- per-core tensors ----
    per_core = []
    for c in range(NC):
        s1, s2 = data1[c], data2[c]
        deg_tot = s1[0]["deg"] + s1[1]["deg"]      # per node position
        node1 = np.minimum(c * NLOC + s1[0]["order"], N - 1)
        node2 = np.minimum(c * NLOC + s2[0]["order"], N - 1)
        xs2 = np.zeros((64, NSLOT), np.float16)
        real2 = s2[0]["order"] < NLOC
        xs2[:, real2] = x[c * NLOC + s2[0]["order"][real2]].T
        per_core.append({
            "gi1_lo": _build_gi(s1[0], sched1[0], 0),
            "gi1_hi": _build_gi(s1[1], sched1[1], HID),
            "gi2_lo": _build_gi(s2[0], sched2[0], 0),
            "gi2_hi": _build_gi(s2[1], sched2[1], HID),
            "mg_hi1": _wrap16(s1[1]["slot_of"][s1[0]["order"]]),
            "mg_hi2": _wrap16(s2[1]["slot_of"][s2[0]["order"]]),
            "posm1": _posm_aug(pos, node1, deg_tot[s1[0]["order"]] == 0),
            "posm2": _posm_aug(pos, node2, deg_tot[s2[0]["order"]] == 0),
            "xsT2": np.ascontiguousarray(xs2),
            "order2_lo": s2[0]["order"],
        })

    # ---- shared tensors ----
    NPAD1 = _ceil(N, CB1) * CB1
    xpT = np.zeros((cfg["CIN"] + 3, NPAD1), np.float16)
    xpT[:cfg["CIN"], :N] = x.T
    xpT[cfg["CIN"]:, :N] = pos.T
    posw = np.empty((3, NC * NSLOT), np.float32)
    for c in range(NC):
        node1 = np.minimum(c * NLOC + data1[c][0]["order"], N - 1)
        posw[:, c * NSLOT:(c + 1) * NSLOT] = pos[node1].T
    shared = {"xpT": np.ascontiguousarray(xpT), "posw": posw}
    return per_core, shared, (sched1, sched2)


def build_bass(cfg, scheds, reps=1):
    import concourse.bass as bass
    import concourse.bacc as bacc
    import concourse.tile as tile
    from concourse import mybir
    from concourse.masks import make_identity
    import contextlib

    N, NC = cfg["N"], cfg["NC"]
    CIN, COUT = cfg["CIN"], cfg["COUT"]
    NLOC = N // NC
    SLOC = _ceil(NLOC, 128)
    NSLOT = SLOC * 128
    LO_LIM, R = cfg["LO_LIM"], cfg["R"]
    HI_R = R - LO_LIM
    NPAD1 = _ceil(N, CB1) * CB1
    sched1, sched2 = scheds
    f32, f16, i16 = mybir.dt.float32, mybir.dt.float16, mybir.dt.int16
    OP = mybir.AluOpType
    AF = mybir.ActivationFunctionType
    AXX = mybir.AxisListType.X
    VC = min(512, NSLOT)          # v-compute chunk (slots)

    nc = bacc.Bacc(num_devices=NC, name="blockconv4", num_swdge_queues=4)

    xpT_in = nc.dram_tensor("xpT", [CIN + 3, NPAD1], f16, kind="ExternalInput")
    xsT2_in = nc.dram_tensor("xsT2", [CIN, NSLOT], f16, kind="ExternalInput")
    posm1_in = nc.dram_tensor("posm1", [5, NSLOT], f16, kind="ExternalInput")
    posm2_in = nc.dram_tensor("posm2", [5, NSLOT], f16, kind="ExternalInput")
    pwps_in = nc.dram_tensor("pwp_self", [COUT, NSLOT], f16,
                             kind="ExternalInput")
    wt = {}
    for nm, shp, dt in (("W1s", [CIN + 3, COUT], f16), ("W1pa", [5, COUT], f16),
                        ("W2as", [COUT, COUT], f16), ("W2pa", [5, COUT], f16),
                        ("Wls", [CIN, COUT], f16),
                        ("g1", [COUT, 1], f32), ("be1", [COUT, 1], f32),
                        ("g2", [COUT, 1], f32), ("be2", [COUT, 1], f32),
                        ("gl", [COUT, 1], f32), ("bel", [COUT, 1], f32)):
        wt[nm] = nc.dram_tensor(nm, shp, dt, kind="ExternalInput")

    W1lo = max(sum(sched1[0]), 1) * 8
    W1hi = max(sum(sched1[1]), 1) * 8
    W2lo = max(sum(sched2[0]), 1) * 8
    W2hi = max(sum(sched2[1]), 1) * 8
    gi_in = {}
    for nm, w in (("gi1_lo", W1lo), ("gi1_hi", W1hi),
                  ("gi2_lo", W2lo), ("gi2_hi", W2hi)):
        gi_in[nm] = nc.dram_tensor(nm, [128, w], i16, kind="ExternalInput")
    mg1_in = nc.dram_tensor("mg_hi1", [128, NSLOT // 16], i16, kind="ExternalInput")
    mg2_in = nc.dram_tensor("mg_hi2", [128, NSLOT // 16], i16, kind="ExternalInput")

    out_t = nc.dram_tensor("out", [COUT, NSLOT], f16, kind="ExternalOutput")

    table_lo = nc.dram_tensor("table_lo", [LO_LIM, COUT], f16)
    table_hi = nc.dram_tensor("table_hi", [HI_R, COUT], f16)
    mbuf = nc.dram_tensor("mbuf", [NSLOT, COUT], f16)
    HAG = min(CB2, NSLOT)         # first AllGather chunk (build-aligned)
    ag_ia = nc.dram_tensor("ag_in_a", [COUT, HAG], f16)
    ag_oa = nc.dram_tensor("ag_out_a", [NC, COUT, HAG], f16,
                           addr_space="Shared")
    if NSLOT > HAG:
        ag_ib = nc.dram_tensor("ag_in_b", [COUT, NSLOT - HAG], f16)
        ag_ob = nc.dram_tensor("ag_out_b", [NC, COUT, NSLOT - HAG], f16,
                               addr_space="Shared")
    else:
        ag_ib = ag_ob = None
    ar_i = nc.dram_tensor("ar_in", [COUT, 4], f32)
    ar_o = nc.dram_tensor("ar_out", [COUT, 4], f32, addr_space="Shared")
    ar2_i = nc.dram_tensor("ar2_in", [COUT, 2], f32)
    ar2_o = nc.dram_tensor("ar2_out", [COUT, 2], f32, addr_space="Shared")
    bar_i = nc.dram_tensor("bar_in", [COUT, 1], f32)
    bar_o = nc.dram_tensor("bar_out", [COUT, 1], f32, addr_space="Shared")
    groups = [list(range(NC))]

    qctr = [0]

    def nextq():
        q = qctr[0] & 3
        qctr[0] += 1
        return q

    with tile.TileContext(nc) as tc:
        ctx = contextlib.ExitStack()
        with ctx:
            sing = ctx.enter_context(tc.tile_pool(name="sing", bufs=1))
            ld = ctx.enter_context(tc.tile_pool(name="ld", bufs=2))
            st = ctx.enter_context(tc.tile_pool(name="st", bufs=2))
            big = ctx.enter_context(tc.tile_pool(name="big", bufs=1))
            ck = ctx.enter_context(tc.tile_pool(name="ck", bufs=2))
            ppb = ctx.enter_context(tc.tile_pool(name="ppb", bufs=4, space="PSUM"))
            pcs = ctx.enter_context(tc.tile_pool(name="pcs", bufs=2, space="PSUM"))
            ppt = ctx.enter_context(tc.tile_pool(name="ppt", bufs=2, space="PSUM"))

            ident = sing.tile([128, 128], f16)
            make_identity(nc, ident)
            negbig = sing.tile([1, COUT], f16)
            nc.vector.memset(negbig[:], BIG_NEG)
            epsv = sing.tile([COUT, 1], f32)
            nc.vector.memset(epsv[:], EPS)

            W = {}
            for nm in ("W1s", "W1pa", "W2as", "W2pa", "Wls"):
                t = sing.tile(list(wt[nm].shape), f16, tag=f"w_{nm}")
                nc.sync.dma_start(t[:], wt[nm][:])
                W[nm] = t
            pvec = {}
            for nm in ("g1", "be1", "g2", "be2", "gl", "bel"):
                v = sing.tile([COUT, 1], f32, tag=f"pv_{nm}")
                nc.sync.dma_start(v[:], wt[nm][:])
                pvec[nm] = v

            mg1 = sing.tile([128, NSLOT // 16], i16, tag="mg1")
            nc.sync.dma_start(mg1[:], mg1_in[:])
            mg2 = sing.tile([128, NSLOT // 16], i16, tag="mg2")
            nc.sync.dma_start(mg2[:], mg2_in[:])

            def table_write_pm(oc, base, G):
                """p-major write: table row base + p*G + g <- oc[p, g, :].
                The lo/hi cut is partition-contiguous, so <=4 DMAs."""
                n = G * 128
                m0 = max(0, min(LO_LIM - base, n))
                pf, rem = divmod(m0, G)
                if pf:
                    d = table_lo[base:base + pf * G, :].rearrange(
                        "(p g) f -> p g f", g=G)
                    nc.scalar.dma_start(d, oc[0:pf, 0:G, :])
                if rem:
                    nc.scalar.dma_start(
                        table_lo[base + pf * G:base + m0, :].rearrange(
                            "(o r) f -> o r f", o=1),
                        oc[pf:pf + 1, 0:rem, :])
                if m0 < n:
                    b2 = base + m0 - LO_LIM
                    if rem:
                        nc.scalar.dma_start(
                            table_hi[b2:b2 + G - rem, :].rearrange(
                                "(o r) f -> o r f", o=1),
                            oc[pf:pf + 1, rem:G, :])
                        b2 += G - rem
                        pf += 1
                    if pf < 128:
                        d = table_hi[b2:b2 + (128 - pf) * G, :].rearrange(
                            "(p g) f -> p g f", g=G)
                        nc.scalar.dma_start(d, oc[pf:128, 0:G, :])

            def side_passes(idxt, sched_side, win, winsz, acc, ro):
                off = 0
                for k, S in enumerate(sched_side):
                    stg = st.tile([128, SLOC, COUT], f16, tag="stage", bufs=2)
                    a = 0
                    while a < S:
                        b = min(a + 8, S)
                        nc.gpsimd.dma_gather(
                            out_ap=stg[:, a:b, :], in_ap=win[0:winsz, :],
                            idxs_ap=idxt[:, off + a * 8:off + b * 8],
                            num_idxs=(b - a) * 128,
                            num_idxs_reg=(b - a) * 128,
                            elem_size=COUT, queue_num=nextq())
                        a = b
                    nc.vector.tensor_tensor(
                        out=acc[:, ro:ro + S, :], in0=acc[:, ro:ro + S, :],
                        in1=stg[:, 0:S, :], op=OP.max)
                    off += S * 8

            def gathers(gi, sched_c, acc, mg):
                """lo then hi passes; mbuf round-trip regather of the hi
                half into lo-slot order."""
                side_passes(gi[0], sched_c[0], table_lo, LO_LIM, acc, 0)
                side_passes(gi[1], sched_c[1], table_hi, HI_R, acc, SLOC)
                nc.sync.dma_start(
                    mbuf[:].rearrange("(s p) f -> p s f", p=128),
                    acc[:, SLOC:2 * SLOC, :])
                hi_lo = st.tile([128, SLOC, COUT], f16, tag="stage", bufs=2)
                a = 0
                while a < SLOC:
                    b = min(a + 8, SLOC)
                    nc.gpsimd.dma_gather(
                        out_ap=hi_lo[:, a:b, :], in_ap=mbuf[:, :],
                        idxs_ap=mg[:, a * 8:b * 8],
                        num_idxs=(b - a) * 128, num_idxs_reg=(b - a) * 128,
                        elem_size=COUT, queue_num=nextq())
                    a = b
                return hi_lo

            def merge_and_v(acc, hi_lo, posm_t, Wp_aug, vT):
                """agg = max(acc lo, hi_lo regather) -> transpose ->
                v^T = agg^T - C'."""
                nc.vector.tensor_tensor(out=hi_lo[:], in0=hi_lo[:],
                                        in1=acc[:, 0:SLOC, :], op=OP.max)
                ssum = [None, None]
                sqq = [None, None]
                nchunk = _ceil(NSLOT, VC)
                for ci in range(nchunk):
                    j0 = ci * VC
                    nr = min(VC, NSLOT - j0) // 128
                    w = nr * 128
                    pt = ppt.tile([128, VC // 128, 128], f16, tag="pt")
                    for r in range(nr):
                        nc.tensor.transpose(
                            out=pt[:, r, :],
                            in_=hi_lo[:, j0 // 128 + r, :],
                            identity=ident[:])
                    ptf = pt[:, 0:nr, :].rearrange("p a b -> p (a b)")
                    aggf = ck.tile([128, VC], f32, tag="aggf")
                    nc.scalar.copy(out=aggf[:, 0:w], in_=ptf)
                    cps = pcs.tile([128, VC], f32, tag="cps")
                    nc.tensor.matmul(out=cps[:, 0:w], lhsT=Wp_aug[:],
                                     rhs=posm_t[:, j0:j0 + w],
                                     start=True, stop=True)
                    nc.vector.tensor_tensor(out=vT[:, j0:j0 + w],
                                            in0=aggf[:, 0:w], in1=cps[:, 0:w],
                                            op=OP.subtract)
                    ps = ck.tile([COUT, 1], f32, tag="ps")
                    nc.vector.tensor_reduce(out=ps[:], in_=vT[:, j0:j0 + w],
                                            op=OP.add, axis=AXX)
                    junk = ck.tile([128, VC], f16, tag="junk")
                    nc.vector.tensor_tensor(out=junk[:, 0:w],
                                            in0=vT[:, j0:j0 + w],
                                            in1=vT[:, j0:j0 + w], op=OP.mult)
                    pq = ck.tile([COUT, 1], f32, tag="pq")
                    nc.vector.tensor_reduce(out=pq[:], in_=junk[:, 0:w],
                                            op=OP.add, axis=AXX)
                    cs = ck.tile([COUT, 1], f32, tag=f"ms{ci & 1}")
                    cq = ck.tile([COUT, 1], f32, tag=f"mq{ci & 1}")
                    if ci == 0:
                        nc.vector.tensor_copy(out=cs[:], in_=ps[:])
                        nc.vector.tensor_copy(out=cq[:], in_=pq[:])
                    else:
                        nc.vector.tensor_tensor(out=cs[:], in0=ps[:],
                                                in1=ssum[(ci - 1) & 1][:],
                                                op=OP.add)
                        nc.vector.tensor_tensor(out=cq[:], in0=pq[:],
                                                in1=sqq[(ci - 1) & 1][:],
                                                op=OP.add)
                    ssum[ci & 1] = cs
                    sqq[ci & 1] = cq
                return ssum[(nchunk - 1) & 1], sqq[(nchunk - 1) & 1]

            def bn_params(sum_ap, sq_ap, g_v, be_v, tagp):
                """scale = g*rsqrt(var+eps), shift = be - mean*scale; [COUT,1]."""
                mean = ck.tile([COUT, 1], f32, tag=f"{tagp}_m")
                nc.vector.tensor_scalar(out=mean[:], in0=sum_ap, scalar1=1.0 / N,
                                        scalar2=None, op0=OP.mult)
                ex2 = ck.tile([COUT, 1], f32, tag=f"{tagp}_e")
                nc.vector.tensor_scalar(out=ex2[:], in0=sq_ap, scalar1=1.0 / N,
                                        scalar2=None, op0=OP.mult)
                m2 = ck.tile([COUT, 1], f32, tag=f"{tagp}_m2")
                nc.vector.tensor_tensor(out=m2[:], in0=mean[:], in1=mean[:],
                                        op=OP.mult)
                var = ck.tile([COUT, 1], f32, tag=f"{tagp}_v")
                nc.vector.tensor_tensor(out=var[:], in0=ex2[:], in1=m2[:],
                                        op=OP.subtract)
                sd = ck.tile([COUT, 1], f32, tag=f"{tagp}_sd")
                nc.scalar.activation(out=sd[:], in_=var[:], func=AF.Sqrt,
                                     bias=epsv[:], scale=1.0)
                rstd = ck.tile([COUT, 1], f32, tag=f"{tagp}_r")
                nc.vector.reciprocal(out=rstd[:], in_=sd[:])
                sc = sing.tile([COUT, 1], f32, tag=f"{tagp}_sc")
                nc.vector.tensor_tensor(out=sc[:], in0=rstd[:], in1=g_v[:],
                                        op=OP.mult)
                ms = ck.tile([COUT, 1], f32, tag=f"{tagp}_ms")
                nc.vector.tensor_tensor(out=ms[:], in0=mean[:], in1=sc[:],
                                        op=OP.mult)
                sh = sing.tile([COUT, 1], f32, tag=f"{tagp}_sh")
                nc.vector.tensor_tensor(out=sh[:], in0=be_v[:], in1=ms[:],
                                        op=OP.subtract)
                return sc, sh

            for _rep in range(reps):
                if _rep == 0:
                    # pre-barrier: overlap cross-core rendezvous with build
                    nc.sync.dma_start(bar_i[:], epsv[:])
                    nc.gpsimd.collective_compute(
                        "AllReduce", OP.add, replica_groups=groups,
                        ins=[bar_i[:]], outs=[bar_o[:]])

                # ---- dummy rows ----
                nc.sync.dma_start(table_lo[0:1, :], negbig[:])
                nc.sync.dma_start(table_hi[HI_R - 1:HI_R, :], negbig[:])

                # ---- conv1 gather index loads ----
                gi1 = {}
                for si, nm in ((0, "gi1_lo"), (1, "gi1_hi")):
                    t = sing.tile([128, max(W1lo, W2lo) if si == 0
                                   else max(W1hi, W2hi)], i16, tag=f"gi_{si}")
                    nc.scalar.dma_start(t[:, 0:gi_in[nm].shape[1]],
                                        gi_in[nm][:])
                    gi1[si] = t

                # ---- conv1 A-table build (p-major batches) ----
                for c0 in range(0, NPAD1, CB1):
                    G = min(CB1 // 128, _ceil(N - c0, 128))
                    lhs = ld.tile([CIN + 3, CB1], f16, tag="lhs1", bufs=3)
                    nc.sync.dma_start(lhs[:], xpT_in[:, c0:c0 + CB1])
                    oc = ld.tile([128, CB1 // 128, COUT], f16, tag="oc", bufs=3)
                    for g2_ in range(_ceil(G, 2)):
                        pb = ppb.tile([128, 2, COUT], f32, tag="pb")
                        for h in range(min(2, G - g2_ * 2)):
                            g = g2_ * 2 + h
                            nc.tensor.matmul(
                                out=pb[:, h, :],
                                lhsT=lhs[:, g * 128:(g + 1) * 128],
                                rhs=W["W1s"][:], start=True, stop=True)
                        nh = min(2, G - g2_ * 2)
                        if g2_ & 1:
                            nc.scalar.copy(out=oc[:, g2_ * 2:g2_ * 2 + nh, :],
                                           in_=pb[:, 0:nh, :])
                        else:
                            nc.vector.tensor_copy(
                                out=oc[:, g2_ * 2:g2_ * 2 + nh, :],
                                in_=pb[:, 0:nh, :])
                    table_write_pm(oc, 1 + c0, G)

                # ---- conv1 gathers ----
                acc = big.tile([128, 2 * SLOC, COUT], f16, tag="acc")
                nc.vector.memset(acc[:], BIG_NEG)
                hl1 = gathers(gi1, sched1, acc, mg1)

                # ---- skip path: skipT = Wl^T @ xsT2 (lo2 order) ----
                skipT = big.tile([COUT, NSLOT], f16, tag="skipT")
                sks = [None, None]
                skq = [None, None]
                nsk = _ceil(NSLOT, VC)
                for ci in range(nsk):
                    j0 = ci * VC
                    hw = min(VC, NSLOT - j0)
                    xs = ld.tile([CIN, VC], f16, tag="lhs2")
                    nc.sync.dma_start(xs[:, 0:hw], xsT2_in[:, j0:j0 + hw])
                    pskip = pcs.tile([128, VC], f32, tag="cps")
                    nc.tensor.matmul(out=pskip[:, 0:hw], lhsT=W["Wls"][:],
                                     rhs=xs[:, 0:hw], start=True, stop=True)
                    nc.scalar.copy(out=skipT[:, j0:j0 + hw],
                                   in_=pskip[:, 0:hw])
                    ps = ck.tile([COUT, 1], f32, tag="ps")
                    nc.vector.tensor_reduce(out=ps[:],
                                            in_=skipT[:, j0:j0 + hw],
                                            op=OP.add, axis=AXX)
                    junk = ck.tile([128, VC], f16, tag="junk")
                    nc.vector.tensor_tensor(out=junk[:, 0:hw],
                                            in0=skipT[:, j0:j0 + hw],
                                            in1=skipT[:, j0:j0 + hw],
                                            op=OP.mult)
                    pq = ck.tile([COUT, 1], f32, tag="pq")
                    nc.vector.tensor_reduce(out=pq[:], in_=junk[:, 0:hw],
                                            op=OP.add, axis=AXX)
                    cs = ck.tile([COUT, 1], f32, tag=f"ss{ci & 1}")
                    cq = ck.tile([COUT, 1], f32, tag=f"sq{ci & 1}")
                    if ci == 0:
                        nc.vector.tensor_copy(out=cs[:], in_=ps[:])
                        nc.vector.tensor_copy(out=cq[:], in_=pq[:])
                    else:
                        nc.vector.tensor_tensor(out=cs[:], in0=ps[:],
                                                in1=sks[(ci - 1) & 1][:],
                                                op=OP.add)
                        nc.vector.tensor_tensor(out=cq[:], in0=pq[:],
                                                in1=skq[(ci - 1) & 1][:],
                                                op=OP.add)
                    sks[ci & 1] = cs
                    skq[ci & 1] = cq
                sksum, sksq = sks[(nsk - 1) & 1], skq[(nsk - 1) & 1]

                # ---- conv1 merge + v1 + stats ----
                posm1 = sing.tile([5, NSLOT], f16, tag="posm")
                nc.sync.dma_start(posm1[:], posm1_in[:])
                pwps = sing.tile([COUT, NSLOT], f16, tag="pwps")
                nc.scalar.dma_start(pwps[:], pwps_in[:])
                v1T = big.tile([COUT, NSLOT], f16, tag="vT")
                s1, q1 = merge_and_v(acc, hl1, posm1, W["W1pa"], v1T)

                # ---- conv2 gather index loads (sync idle here) ----
                gi2 = {}
                for si, nm in ((0, "gi2_lo"), (1, "gi2_hi")):
                    t = sing.tile([128, max(W1lo, W2lo) if si == 0
                                   else max(W1hi, W2hi)], i16, tag=f"gi_{si}")
                    nc.sync.dma_start(t[:, 0:gi_in[nm].shape[1]], gi_in[nm][:])
                    gi2[si] = t

                arst = sing.tile([COUT, 4], f32, tag="arst")
                nc.vector.tensor_copy(out=arst[:, 0:1], in_=s1[:])
                nc.vector.tensor_copy(out=arst[:, 1:2], in_=q1[:])
                nc.vector.tensor_copy(out=arst[:, 2:3], in_=sksum[:])
                nc.vector.tensor_copy(out=arst[:, 3:4], in_=sksq[:])
                nc.sync.dma_start(ar_i[:], arst[:])
                nc.gpsimd.collective_compute(
                    "AllReduce", OP.add, replica_groups=groups,
                    ins=[ar_i[:]], outs=[ar_o[:]])
                arres = sing.tile([COUT, 4], f32, tag="arres")
                nc.sync.dma_start(arres[:], ar_o[:])

                sc1, sh1 = bn_params(arres[:, 0:1], arres[:, 1:2],
                                     pvec["g1"], pvec["be1"], "bn1")
                scl, shl = bn_params(arres[:, 2:3], arres[:, 3:4],
                                     pvec["gl"], pvec["bel"], "bnl")

                # h' = relu(bn1(v1)) + pos@P, once per core (in place over
                # v1T), then AllGathered
                nc.scalar.activation(out=v1T[:], in_=v1T[:], func=AF.Relu,
                                     bias=sh1[:], scale=sc1[:])
                nc.vector.tensor_tensor(out=v1T[:], in0=v1T[:], in1=pwps[:],
                                        op=OP.add)
                if ag_ib is not None:
                    nc.sync.dma_start(ag_ia[:], v1T[:, 0:HAG])
                    nc.gpsimd.collective_compute(
                        "AllGather", OP.bypass, replica_groups=groups,
                        ins=[ag_ia[:]], outs=[ag_oa[:]])
                    nc.sync.dma_start(ag_ib[:], v1T[:, HAG:NSLOT])
                    nc.gpsimd.collective_compute(
                        "AllGather", OP.bypass, replica_groups=groups,
                        ins=[ag_ib[:]], outs=[ag_ob[:]])
                else:
                    nc.sync.dma_start(ag_ia[:], v1T[:])
                    nc.gpsimd.collective_compute(
                        "AllGather", OP.bypass, replica_groups=groups,
                        ins=[ag_ia[:]], outs=[ag_oa[:]])

                # ---- conv2 A-table build (p-major) ----
                for ct in range(NC):
                    for c0 in range(0, NSLOT, CB2):
                        w = min(CB2, NSLOT - c0)
                        nw = w // 128
                        lhs = ld.tile([COUT, CB2], f16, tag="vstg")
                        if c0 < HAG:
                            nc.sync.dma_start(lhs[:, 0:w],
                                              ag_oa[ct, :, c0:c0 + w])
                        else:
                            nc.sync.dma_start(
                                lhs[:, 0:w],
                                ag_ob[ct, :, c0 - HAG:c0 - HAG + w])
                        oc = ld.tile([128, CB2 // 128, COUT], f16, tag="oc2")
                        for g2_ in range(_ceil(nw, 2)):
                            pb = ppb.tile([128, 2, COUT], f32, tag="pb")
                            for h in range(min(2, nw - g2_ * 2)):
                                g = g2_ * 2 + h
                                nc.tensor.matmul(
                                    out=pb[:, h, :],
                                    lhsT=lhs[:, g * 128:(g + 1) * 128],
                                    rhs=W["W2as"][:], start=True, stop=True)
                            nh = min(2, nw - g2_ * 2)
                            if g2_ & 1:
                                nc.scalar.copy(
                                    out=oc[:, g2_ * 2:g2_ * 2 + nh, :],
                                    in_=pb[:, 0:nh, :])
                            else:
                                nc.vector.tensor_copy(
                                    out=oc[:, g2_ * 2:g2_ * 2 + nh, :],
                                    in_=pb[:, 0:nh, :])
                        table_write_pm(oc, 1 + ct * NSLOT + c0, nw)

                # ---- precompute bnl(skip) while conv2 gathers run ----
                bskT = big.tile([COUT, NSLOT], f16, tag="skip2")
                for j0 in range(0, NSLOT, VC):
                    w = min(VC, NSLOT - j0)
                    nc.scalar.activation(out=bskT[:, j0:j0 + w],
                                         in_=skipT[:, j0:j0 + w],
                                         func=AF.Identity,
                                         bias=shl[:], scale=scl[:])

                # ---- conv2 gathers ----
                acc2 = big.tile([128, 2 * SLOC, COUT], f16, tag="acc")
                nc.vector.memset(acc2[:], BIG_NEG)
                hl2 = gathers(gi2, sched2, acc2, mg2)

                # ---- conv2 merge + v2 + stats ----
                posm2 = sing.tile([5, NSLOT], f16, tag="posm")
                nc.sync.dma_start(posm2[:], posm2_in[:])
                v2T = big.tile([COUT, NSLOT], f16, tag="vT")
                s2, q2 = merge_and_v(acc2, hl2, posm2, W["W2pa"], v2T)

                arst2 = sing.tile([COUT, 2], f32, tag="arst2")
                nc.vector.tensor_copy(out=arst2[:, 0:1], in_=s2[:])
                nc.vector.tensor_copy(out=arst2[:, 1:2], in_=q2[:])
                nc.sync.dma_start(ar2_i[:], arst2[:])
                nc.gpsimd.collective_compute(
                    "AllReduce", OP.add, replica_groups=groups,
                    ins=[ar2_i[:]], outs=[ar2_o[:]])
                arres2 = sing.tile([COUT, 2], f32, tag="arres2")
                nc.sync.dma_start(arres2[:], ar2_o[:])
                sc2, sh2 = bn_params(arres2[:, 0:1], arres2[:, 1:2],
                                     pvec["g2"], pvec["be2"], "bn2")

                # ---- final = relu(bn2(v2) + bnl(skip)) ----
                for j0 in range(0, NSLOT, VC):
                    w = min(VC, NSLOT - j0)
                    a_ = ck.tile([128, VC], f32, tag="aggf")
                    nc.scalar.activation(out=a_[:, 0:w],
                                         in_=v2T[:, j0:j0 + w],
                                         func=AF.Identity,
                                         bias=sh2[:], scale=sc2[:])
                    fin = ck.tile([128, VC], f16, tag="fin")
                    nc.vector.tensor_tensor(out=fin[:, 0:w], in0=a_[:, 0:w],
                                            in1=bskT[:, j0:j0 + w], op=OP.add)
                    nc.vector.tensor_scalar(out=fin[:, 0:w], in0=fin[:, 0:w],
                                            scalar1=0.0, scalar2=None,
                                            op0=OP.max)
                    nc.sync.dma_start(out_t[:, j0:j0 + w], fin[:, 0:w])

    nc.compile()
    return nc


def make_in_maps(inputs, cfg, per_core, shared):
    f16 = np.float16
    CIN, COUT = cfg["CIN"], cfg["COUT"]
    W1 = np.asarray(inputs["W1"], np.float32)
    b1 = np.asarray(inputs["b1"], np.float32)
    W2 = np.asarray(inputs["W2"], np.float32)
    b2 = np.asarray(inputs["b2"], np.float32)
    ones = np.ones((1, COUT), np.float32)
    W1pa = np.concatenate([W1[CIN:CIN + 3, :], b1[None, :], ones], axis=0)
    W2pa = np.concatenate([W2[COUT:COUT + 3, :], b2[None, :], ones], axis=0)
    W2a = W2[0:COUT, :].astype(np.float64)
    W2p = W2[COUT:COUT + 3, :].astype(np.float64)
    P = np.linalg.solve(W2a.T, W2p.T).T          # P @ W2a == W2p
    poswP = (P.T @ shared["posw"].astype(np.float64)).astype(f16)
    NSLOT = poswP.shape[1] // len(per_core)
    base = dict(
        xpT=shared["xpT"],
        W1s=W1.astype(f16), W1pa=W1pa.astype(f16),
        W2as=W2[0:COUT, :].astype(f16), W2pa=W2pa.astype(f16),
        Wls=np.asarray(inputs["Wl"], np.float32).astype(f16),
        g1=np.asarray(inputs["g1"], np.float32).reshape(-1, 1),
        be1=np.asarray(inputs["be1"], np.float32).reshape(-1, 1),
        g2=np.asarray(inputs["g2"], np.float32).reshape(-1, 1),
        be2=np.asarray(inputs["be2"], np.float32).reshape(-1, 1),
        gl=np.asarray(inputs["gl"], np.float32).reshape(-1, 1),
        bel=np.asarray(inputs["bel"], np.float32).reshape(-1, 1),
    )
    in_maps = []
    for c, pc in enumerate(per_core):
        m = dict(base)
        for k in ("gi1_lo", "gi1_hi", "gi2_lo", "gi2_hi",
                  "mg_hi1", "mg_hi2", "posm1", "posm2", "xsT2"):
            m[k] = pc[k]
        m["pwp_self"] = np.ascontiguousarray(
            poswP[:, c * NSLOT:(c + 1) * NSLOT])
        in_maps.append(m)
    return in_maps


_CACHE = {}


def run(inputs, cfg, use_sim=False, trace=False):
    per_core, shared, scheds = host_prep(
        inputs["edge_index"], inputs["pos"], inputs["x"], cfg)
    key = (cfg["N"], tuple(scheds[0][0]), tuple(scheds[0][1]),
           tuple(scheds[1][0]), tuple(scheds[1][1]))
    if key not in _CACHE:
        _CACHE[key] = build_bass(cfg, scheds)
    nc = _CACHE[key]
    in_maps = make_in_maps(inputs, cfg, per_core, shared)
    NC = cfg["NC"]
    NLOC = cfg["N"] // NC
    if use_sim:
        from concourse.bass_interp import MultiCoreSim
        sim = MultiCoreSim(nc, num_cores=NC, require_finite=False,
                           require_nnan=False)
        for c in range(NC):
            for k, v in in_maps[c].items():
                sim.cores[c].tensor(k)[:] = v
        sim.simulate(check_with_hw=False)
        outs = [np.array(sim.cores[c].tensor("out")) for c in range(NC)]
        res = None
    else:
        from concourse.bass_utils import run_bass_kernel_spmd
        res = run_bass_kernel_spmd(nc, in_maps, core_ids=list(range(NC)),
                                   trace=trace)
        outs = [res.results[c]["out"] for c in range(NC)]
    full = np.empty((cfg["N"], cfg["COUT"]), np.float32)
    for c in range(NC):
        order2 = per_core[c]["order2_lo"]
        real = order2 < NLOC
        full[c * NLOC + order2[real]] = outs[c].T[real].astype(np.float32)
    return full, res


def kernel(**inputs):
    out, _ = run(inputs, FULL_CFG, use_sim=False)
    return out


# revision 28
# speedup vs baseline: 1.0662x; 1.0662x over previous
"""Trainium2 Bass kernel for nn_BlockConv (PointNet-style GNN block), 8 cores.

Algebraic core: msg_e = concat(x_src, pos_src-pos_dst) @ W + b
  = A[src] - C[dst], with A = concat(x,pos)@W (per-node table) and
  C = pos@W[-3:] - b (per-dst, constant within a segment).
  segment_max over dst = (gather+max of A rows) - C[dst]. Memory-bound.

Distribution: dst-sharded; per-core edge gathers from a replicated fp16
A-table in HBM via 4-queue SWDGE dma_gather (descriptor-rate limited:
~3ns/row on 4 queues vs ~9 on one). Tables, stages, h are fp16. Host
pre-transposes x/pos so table builds are straight fp16 matmuls; the
table uses a p-major row layout (row = base + p*G + g for batch column
p + 128*g) so each table write is one contiguous-per-partition DMA
instead of a 2048-descriptor strided store. Dst slots are degree-sorted
per side (lo/hi row windows for int16 idx) so gather pass k covers a
slot prefix. Dead (degree-0) slots get an exact v=0 via a 5th posm row
carrying -BIG through the C-matmul. The AllGather ships pre-BN v1^T in
two column chunks (first issued mid-merge) concurrently with the stats
AllReduce; conv2's build applies BN+relu on the fly (DVE) and folds
pos@W2p into h via host-computed P = W2p @ W2a^-1. Output is
feature-major in lo2-slot order; the host unpermutes.
"""
import sys
import numpy as np

if "/opt/trn_rl_repo" not in sys.path:
    sys.path.insert(0, "/opt/trn_rl_repo")

BIG_NEG = -60000.0
EPS = 1e-5
CB1 = 2048          # conv1 table build batch (columns)
CB2 = 2048          # conv2 table build batch (columns)

FULL_CFG = dict(N=50000, E=800000, CIN=64, COUT=128, NC=8,
                LO_LIM=32768, R=50432)
MINI_CFG = dict(N=2048, E=16384, CIN=64, COUT=128, NC=8,
                LO_LIM=1024, R=2432)
MID_CFG = dict(N=16384, E=262144, CIN=64, COUT=128, NC=8,
               LO_LIM=8192, R=16768)


def _ceil(a, b):
    return (a + b - 1) // b


def _wrap16(ids):
    """flat int list (len % 128 == 0) -> [128, len//16] int16 wrapped:
    unwrapped[j] = g[j%16, j//16], replicated over the 8 core groups."""
    a = np.asarray(ids, np.int64)
    assert a.size % 128 == 0 and a.min() >= 0 and a.max() < 32768
    g = a.reshape(a.size // 16, 16).T.astype(np.int16)   # [16, L/16]
    return np.tile(g, (8, 1))                            # [128, L/16]


def _row1(n, N):
    """p-major conv1 table row for node n (batches of CB1 columns)."""
    b = n // CB1
    o = n - b * CB1
    G = np.minimum(CB1 // 128, (N - b * CB1 + 127) // 128)
    return 1 + b * CB1 + (o % 128) * G + o // 128


def _row2(ct, slot, NSLOT):
    """p-major conv2 table row for (core ct, lo1-slot)."""
    c0 = (slot // CB2) * CB2
    w = np.minimum(CB2, NSLOT - c0)
    o = slot - c0
    return 1 + ct * NSLOT + c0 + (o % 128) * (w // 128) + o // 128


def _side_prep(rows, d_loc, NSLOT):
    """Degree-sort dst slots for one source window. rows = side-local table
    row ids per edge; d_loc = local dst node position per edge."""
    deg = np.bincount(d_loc, minlength=NSLOT)
    order = np.argsort(-deg, kind="stable")        # slot s -> node position
    slot_of = np.empty(NSLOT, np.int64)            # node position -> slot
    slot_of[order] = np.arange(NSLOT)
    isort = np.argsort(d_loc, kind="stable")
    starts = np.zeros(NSLOT + 1, np.int64)
    np.cumsum(deg, out=starts[1:])
    return {"deg": deg, "order": order, "slot_of": slot_of,
            "s_sorted": rows[isort], "starts": starts,
            "cnts": np.sort(deg)[::-1]}


def _build_sched(sides_by_core):
    """sched[si] = list of S_k (128-slot chunks) per pass, max across cores."""
    sched = []
    for si in range(2):
        Sk = []
        kmax = max(int(sd[si]["cnts"][0]) for sd in sides_by_core)
        for k in range(kmax):
            cnt = max(int((sd[si]["cnts"] > k).sum()) for sd in sides_by_core)
            if cnt == 0:
                break
            Sk.append(_ceil(cnt, 128))
        sched.append(Sk)
    return sched


def _build_gi(side, sched_side, dummy):
    blocks = []
    for k, S in enumerate(sched_side):
        L = S * 128
        ids = np.full(L, dummy, np.int64)
        nsl = int((side["cnts"] > k).sum())
        nodes = side["order"][:nsl]
        ids[:nsl] = side["s_sorted"][side["starts"][nodes] + k]
        blocks.append(_wrap16(ids))
    if not blocks:
        return np.zeros((128, 8), np.int16)
    return np.concatenate(blocks, axis=1)


def _posm_aug(pos, node, dead):
    """[5, NSLOT] fp16: rows 0-2 pos, row 3 = -1 (bias), row 4 = -BIG flag.
    Dead slots get rows 0-3 zeroed and row4=-BIG so C' = -BIG exactly and
    v = agg - C' = 0 exactly."""
    m = np.empty((5, node.size), np.float16)
    m[:3] = pos[node].T
    m[3] = -1.0
    m[4] = 0.0
    m[0:4, dead] = 0.0
    m[4, dead] = BIG_NEG
    return np.ascontiguousarray(m)


def host_prep(edge_index, pos, x, cfg):
    N, NC, LO_LIM, R = cfg["N"], cfg["NC"], cfg["LO_LIM"], cfg["R"]
    NLOC = N // NC
    SLOC = _ceil(NLOC, 128)
    NSLOT = SLOC * 128
    HID = R - 1 - LO_LIM          # hi-local dummy row (global row R-1)
    src = np.asarray(edge_index[0], np.int64)
    dst = np.asarray(edge_index[1], np.int64)
    core_of = dst // NLOC
    pos = np.asarray(pos, np.float32)
    x = np.asarray(x, np.float32)

    # ---- conv1: per-core lo/hi sides in p-major row space ----
    r1 = _row1(src, N)
    data1, slot1_lo_glob = [], np.empty(N, np.int64)
    for c in range(NC):
        m = core_of == c
        r = r1[m]
        d = dst[m] - c * NLOC
        sides = []
        for si, sel in ((0, r < LO_LIM), (1, r >= LO_LIM)):
            sides.append(_side_prep(r[sel] - (LO_LIM if si else 0),
                                    d[sel], NSLOT))
        data1.append(sides)
        slot1_lo_glob[c * NLOC:(c + 1) * NLOC] = sides[0]["slot_of"][:NLOC]
    sched1 = _build_sched(data1)

    # ---- conv2: rows in p-major (core, lo1-slot) space ----
    r2 = _row2(src // NLOC, slot1_lo_glob[src], NSLOT)
    data2 = []
    for c in range(NC):
        m = core_of == c
        r = r2[m]
        d = dst[m] - c * NLOC
        sides = []
        for si, sel in ((0, r < LO_LIM), (1, r >= LO_LIM)):
            sides.append(_side_prep(r[sel] - (LO_LIM if si else 0),
                                    d[sel], NSLOT))
        data2.append(sides)
    sched2 = _build_sched(data2)

    # ---- per-core tensors ----
    per_core = []
    for c in range(NC):
        s1, s2 = data1[c], data2[c]
        deg_tot = s1[0]["deg"] + s1[1]["deg"]      # per node position
        node1 = np.minimum(c * NLOC + s1[0]["order"], N - 1)
        node2 = np.minimum(c * NLOC + s2[0]["order"], N - 1)
        xs2 = np.zeros((64, NSLOT), np.float16)
        real2 = s2[0]["order"] < NLOC
        xs2[:, real2] = x[c * NLOC + s2[0]["order"][real2]].T
        per_core.append({
            "gi1_lo": _build_gi(s1[0], sched1[0], 0),
            "gi1_hi": _build_gi(s1[1], sched1[1], HID),
            "gi2_lo": _build_gi(s2[0], sched2[0], 0),
            "gi2_hi": _build_gi(s2[1], sched2[1], HID),
            "mg_hi1": _wrap16(s1[1]["slot_of"][s1[0]["order"]]),
            "mg_hi2": _wrap16(s2[1]["slot_of"][s2[0]["order"]]),
            "posm1": _posm_aug(pos, node1, deg_tot[s1[0]["order"]] == 0),
            "posm2": _posm_aug(pos, node2, deg_tot[s2[0]["order"]] == 0),
            "xsT2": np.ascontiguousarray(xs2),
            "order2_lo": s2[0]["order"],
        })

    # ---- shared tensors ----
    NPAD1 = _ceil(N, CB1) * CB1
    xpT = np.zeros((cfg["CIN"] + 3, NPAD1), np.float16)
    xpT[:cfg["CIN"], :N] = x.T
    xpT[cfg["CIN"]:, :N] = pos.T
    posw = np.empty((3, NC * NSLOT), np.float32)
    for c in range(NC):
        node1 = np.minimum(c * NLOC + data1[c][0]["order"], N - 1)
        posw[:, c * NSLOT:(c + 1) * NSLOT] = pos[node1].T
    shared = {"xpT": np.ascontiguousarray(xpT), "posw": posw}
    return per_core, shared, (sched1, sched2)


def build_bass(cfg, scheds, reps=1):
    import concourse.bass as bass
    import concourse.bacc as bacc
    import concourse.tile as tile
    from concourse import mybir
    from concourse.masks import make_identity
    import contextlib

    N, NC = cfg["N"], cfg["NC"]
    CIN, COUT = cfg["CIN"], cfg["COUT"]
    NLOC = N // NC
    SLOC = _ceil(NLOC, 128)
    NSLOT = SLOC * 128
    LO_LIM, R = cfg["LO_LIM"], cfg["R"]
    HI_R = R - LO_LIM
    NPAD1 = _ceil(N, CB1) * CB1
    sched1, sched2 = scheds
    f32, f16, i16 = mybir.dt.float32, mybir.dt.float16, mybir.dt.int16
    OP = mybir.AluOpType
    AF = mybir.ActivationFunctionType
    AXX = mybir.AxisListType.X
    VC = min(512, NSLOT)          # v-compute chunk (slots)

    nc = bacc.Bacc(num_devices=NC, name="blockconv4", num_swdge_queues=4)

    xpT_in = nc.dram_tensor("xpT", [CIN + 3, NPAD1], f16, kind="ExternalInput")
    xsT2_in = nc.dram_tensor("xsT2", [CIN, NSLOT], f16, kind="ExternalInput")
    posm1_in = nc.dram_tensor("posm1", [5, NSLOT], f16, kind="ExternalInput")
    posm2_in = nc.dram_tensor("posm2", [5, NSLOT], f16, kind="ExternalInput")
    pwps_in = nc.dram_tensor("pwp_self", [COUT, NSLOT], f16,
                             kind="ExternalInput")
    wt = {}
    for nm, shp, dt in (("W1s", [CIN + 3, COUT], f16), ("W1pa", [5, COUT], f16),
                        ("W2as", [COUT, COUT], f16), ("W2pa", [5, COUT], f16),
                        ("Wls", [CIN, COUT], f16),
                        ("g1", [COUT, 1], f32), ("be1", [COUT, 1], f32),
                        ("g2", [COUT, 1], f32), ("be2", [COUT, 1], f32),
                        ("gl", [COUT, 1], f32), ("bel", [COUT, 1], f32)):
        wt[nm] = nc.dram_tensor(nm, shp, dt, kind="ExternalInput")

    W1lo = max(sum(sched1[0]), 1) * 8
    W1hi = max(sum(sched1[1]), 1) * 8
    W2lo = max(sum(sched2[0]), 1) * 8
    W2hi = max(sum(sched2[1]), 1) * 8
    gi_in = {}
    for nm, w in (("gi1_lo", W1lo), ("gi1_hi", W1hi),
                  ("gi2_lo", W2lo), ("gi2_hi", W2hi)):
        gi_in[nm] = nc.dram_tensor(nm, [128, w], i16, kind="ExternalInput")
    mg1_in = nc.dram_tensor("mg_hi1", [128, NSLOT // 16], i16, kind="ExternalInput")
    mg2_in = nc.dram_tensor("mg_hi2", [128, NSLOT // 16], i16, kind="ExternalInput")

    out_t = nc.dram_tensor("out", [COUT, NSLOT], f16, kind="ExternalOutput")

    table_lo = nc.dram_tensor("table_lo", [LO_LIM, COUT], f16)
    table_hi = nc.dram_tensor("table_hi", [HI_R, COUT], f16)
    mbuf = nc.dram_tensor("mbuf", [NSLOT, COUT], f16)
    HAG = min(CB2, NSLOT)         # first AllGather chunk (build-aligned)
    ag_ia = nc.dram_tensor("ag_in_a", [COUT, HAG], f16)
    ag_oa = nc.dram_tensor("ag_out_a", [NC, COUT, HAG], f16,
                           addr_space="Shared")
    if NSLOT > HAG:
        ag_ib = nc.dram_tensor("ag_in_b", [COUT, NSLOT - HAG], f16)
        ag_ob = nc.dram_tensor("ag_out_b", [NC, COUT, NSLOT - HAG], f16,
                               addr_space="Shared")
    else:
        ag_ib = ag_ob = None
    ar_i = nc.dram_tensor("ar_in", [COUT, 4], f32)
    ar_o = nc.dram_tensor("ar_out", [COUT, 4], f32, addr_space="Shared")
    ar2_i = nc.dram_tensor("ar2_in", [COUT, 2], f32)
    ar2_o = nc.dram_tensor("ar2_out", [COUT, 2], f32, addr_space="Shared")
    bar_i = nc.dram_tensor("bar_in", [COUT, 1], f32)
    bar_o = nc.dram_tensor("bar_out", [COUT, 1], f32, addr_space="Shared")
    groups = [list(range(NC))]

    qctr = [0]

    def nextq():
        q = qctr[0] & 3
        qctr[0] += 1
        return q

    with tile.TileContext(nc) as tc:
        ctx = contextlib.ExitStack()
        with ctx:
            sing = ctx.enter_context(tc.tile_pool(name="sing", bufs=1))
            ld = ctx.enter_context(tc.tile_pool(name="ld", bufs=2))
            st = ctx.enter_context(tc.tile_pool(name="st", bufs=2))
            big = ctx.enter_context(tc.tile_pool(name="big", bufs=1))
            ck = ctx.enter_context(tc.tile_pool(name="ck", bufs=2))
            ppb = ctx.enter_context(tc.tile_pool(name="ppb", bufs=4, space="PSUM"))
            pcs = ctx.enter_context(tc.tile_pool(name="pcs", bufs=2, space="PSUM"))
            ppt = ctx.enter_context(tc.tile_pool(name="ppt", bufs=2, space="PSUM"))

            ident = sing.tile([128, 128], f16)
            make_identity(nc, ident)
            negbig = sing.tile([1, COUT], f16)
            nc.vector.memset(negbig[:], BIG_NEG)
            epsv = sing.tile([COUT, 1], f32)
            nc.vector.memset(epsv[:], EPS)

            W = {}
            for nm in ("W1s", "W1pa", "W2as", "W2pa", "Wls"):
                t = sing.tile(list(wt[nm].shape), f16, tag=f"w_{nm}")
                nc.sync.dma_start(t[:], wt[nm][:])
                W[nm] = t
            pvec = {}
            for nm in ("g1", "be1", "g2", "be2", "gl", "bel"):
                v = sing.tile([COUT, 1], f32, tag=f"pv_{nm}")
                nc.sync.dma_start(v[:], wt[nm][:])
                pvec[nm] = v

            mg1 = sing.tile([128, NSLOT // 16], i16, tag="mg1")
            nc.sync.dma_start(mg1[:], mg1_in[:])
            mg2 = sing.tile([128, NSLOT // 16], i16, tag="mg2")
            nc.sync.dma_start(mg2[:], mg2_in[:])

            def table_write_pm(oc, base, G):
                """p-major write: table row base + p*G + g <- oc[p, g, :].
                The lo/hi cut is partition-contiguous, so <=4 DMAs."""
                n = G * 128
                m0 = max(0, min(LO_LIM - base, n))
                pf, rem = divmod(m0, G)
                if pf:
                    d = table_lo[base:base + pf * G, :].rearrange(
                        "(p g) f -> p g f", g=G)
                    nc.scalar.dma_start(d, oc[0:pf, 0:G, :])
                if rem:
                    nc.scalar.dma_start(
                        table_lo[base + pf * G:base + m0, :].rearrange(
                            "(o r) f -> o r f", o=1),
                        oc[pf:pf + 1, 0:rem, :])
                if m0 < n:
                    b2 = base + m0 - LO_LIM
                    if rem:
                        nc.scalar.dma_start(
                            table_hi[b2:b2 + G - rem, :].rearrange(
                                "(o r) f -> o r f", o=1),
                            oc[pf:pf + 1, rem:G, :])
                        b2 += G - rem
                        pf += 1
                    if pf < 128:
                        d = table_hi[b2:b2 + (128 - pf) * G, :].rearrange(
                            "(p g) f -> p g f", g=G)
                        nc.scalar.dma_start(d, oc[pf:128, 0:G, :])

            def side_passes(idxt, sched_side, win, winsz, acc, ro):
                off = 0
                for k, S in enumerate(sched_side):
                    stg = st.tile([128, SLOC, COUT], f16, tag="stage", bufs=3)
                    a = 0
                    while a < S:
                        b = min(a + 8, S)
                        nc.gpsimd.dma_gather(
                            out_ap=stg[:, a:b, :], in_ap=win[0:winsz, :],
                            idxs_ap=idxt[:, off + a * 8:off + b * 8],
                            num_idxs=(b - a) * 128,
                            num_idxs_reg=(b - a) * 128,
                            elem_size=COUT, queue_num=nextq())
                        a = b
                    nc.vector.tensor_tensor(
                        out=acc[:, ro:ro + S, :], in0=acc[:, ro:ro + S, :],
                        in1=stg[:, 0:S, :], op=OP.max)
                    off += S * 8

            def gathers(gi, sched_c, acc, mg):
                """lo then hi passes; mbuf round-trip regather of the hi
                half into lo-slot order."""
                side_passes(gi[0], sched_c[0], table_lo, LO_LIM, acc, 0)
                side_passes(gi[1], sched_c[1], table_hi, HI_R, acc, SLOC)
                nc.sync.dma_start(
                    mbuf[:].rearrange("(s p) f -> p s f", p=128),
                    acc[:, SLOC:2 * SLOC, :])
                hi_lo = st.tile([128, SLOC, COUT], f16, tag="stage", bufs=3)
                a = 0
                while a < SLOC:
                    b = min(a + 8, SLOC)
                    nc.gpsimd.dma_gather(
                        out_ap=hi_lo[:, a:b, :], in_ap=mbuf[:, :],
                        idxs_ap=mg[:, a * 8:b * 8],
                        num_idxs=(b - a) * 128, num_idxs_reg=(b - a) * 128,
                        elem_size=COUT, queue_num=nextq())
                    a = b
                return hi_lo

            def merge_and_v(acc, hi_lo, posm_t, Wp_aug, vT):
                """agg = max(acc lo, hi_lo regather) -> transpose ->
                v^T = agg^T - C'."""
                nc.vector.tensor_tensor(out=hi_lo[:], in0=hi_lo[:],
                                        in1=acc[:, 0:SLOC, :], op=OP.max)
                ssum = [None, None]
                sqq = [None, None]
                nchunk = _ceil(NSLOT, VC)
                for ci in range(nchunk):
                    j0 = ci * VC
                    nr = min(VC, NSLOT - j0) // 128
                    w = nr * 128
                    pt = ppt.tile([128, VC // 128, 128], f16, tag="pt")
                    for r in range(nr):
                        nc.tensor.transpose(
                            out=pt[:, r, :],
                            in_=hi_lo[:, j0 // 128 + r, :],
                            identity=ident[:])
                    ptf = pt[:, 0:nr, :].rearrange("p a b -> p (a b)")
                    aggf = ck.tile([128, VC], f32, tag="aggf")
                    nc.scalar.copy(out=aggf[:, 0:w], in_=ptf)
                    cps = pcs.tile([128, VC], f32, tag="cps")
                    nc.tensor.matmul(out=cps[:, 0:w], lhsT=Wp_aug[:],
                                     rhs=posm_t[:, j0:j0 + w],
                                     start=True, stop=True)
                    nc.vector.tensor_tensor(out=vT[:, j0:j0 + w],
                                            in0=aggf[:, 0:w], in1=cps[:, 0:w],
                                            op=OP.subtract)
                    ps = ck.tile([COUT, 1], f32, tag="ps")
                    nc.vector.tensor_reduce(out=ps[:], in_=vT[:, j0:j0 + w],
                                            op=OP.add, axis=AXX)
                    junk = ck.tile([128, VC], f16, tag="junk")
                    nc.vector.tensor_tensor(out=junk[:, 0:w],
                                            in0=vT[:, j0:j0 + w],
                                            in1=vT[:, j0:j0 + w], op=OP.mult)
                    pq = ck.tile([COUT, 1], f32, tag="pq")
                    nc.vector.tensor_reduce(out=pq[:], in_=junk[:, 0:w],
                                            op=OP.add, axis=AXX)
                    cs = ck.tile([COUT, 1], f32, tag=f"ms{ci & 1}")
                    cq = ck.tile([COUT, 1], f32, tag=f"mq{ci & 1}")
                    if ci == 0:
                        nc.vector.tensor_copy(out=cs[:], in_=ps[:])
                        nc.vector.tensor_copy(out=cq[:], in_=pq[:])
                    else:
                        nc.vector.tensor_tensor(out=cs[:], in0=ps[:],
                                                in1=ssum[(ci - 1) & 1][:],
                                                op=OP.add)
                        nc.vector.tensor_tensor(out=cq[:], in0=pq[:],
                                                in1=sqq[(ci - 1) & 1][:],
                                                op=OP.add)
                    ssum[ci & 1] = cs
                    sqq[ci & 1] = cq
                return ssum[(nchunk - 1) & 1], sqq[(nchunk - 1) & 1]

            def bn_params(sum_ap, sq_ap, g_v, be_v, tagp):
                """scale = g*rsqrt(var+eps), shift = be - mean*scale; [COUT,1]."""
                mean = ck.tile([COUT, 1], f32, tag=f"{tagp}_m")
                nc.vector.tensor_scalar(out=mean[:], in0=sum_ap, scalar1=1.0 / N,
                                        scalar2=None, op0=OP.mult)
                ex2 = ck.tile([COUT, 1], f32, tag=f"{tagp}_e")
                nc.vector.tensor_scalar(out=ex2[:], in0=sq_ap, scalar1=1.0 / N,
                                        scalar2=None, op0=OP.mult)
                m2 = ck.tile([COUT, 1], f32, tag=f"{tagp}_m2")
                nc.vector.tensor_tensor(out=m2[:], in0=mean[:], in1=mean[:],
                                        op=OP.mult)
                var = ck.tile([COUT, 1], f32, tag=f"{tagp}_v")
                nc.vector.tensor_tensor(out=var[:], in0=ex2[:], in1=m2[:],
                                        op=OP.subtract)
                sd = ck.tile([COUT, 1], f32, tag=f"{tagp}_sd")
                nc.scalar.activation(out=sd[:], in_=var[:], func=AF.Sqrt,
                                     bias=epsv[:], scale=1.0)
                rstd = ck.tile([COUT, 1], f32, tag=f"{tagp}_r")
                nc.vector.reciprocal(out=rstd[:], in_=sd[:])
                sc = sing.tile([COUT, 1], f32, tag=f"{tagp}_sc")
                nc.vector.tensor_tensor(out=sc[:], in0=rstd[:], in1=g_v[:],
                                        op=OP.mult)
                ms = ck.tile([COUT, 1], f32, tag=f"{tagp}_ms")
                nc.vector.tensor_tensor(out=ms[:], in0=mean[:], in1=sc[:],
                                        op=OP.mult)
                sh = sing.tile([COUT, 1], f32, tag=f"{tagp}_sh")
                nc.vector.tensor_tensor(out=sh[:], in0=be_v[:], in1=ms[:],
                                        op=OP.subtract)
                return sc, sh

            for _rep in range(reps):
                if _rep == 0:
                    # pre-barrier: overlap cross-core rendezvous with build
                    nc.sync.dma_start(bar_i[:], epsv[:])
                    nc.gpsimd.collective_compute(
                        "AllReduce", OP.add, replica_groups=groups,
                        ins=[bar_i[:]], outs=[bar_o[:]])

                # ---- dummy rows ----
                nc.sync.dma_start(table_lo[0:1, :], negbig[:])
                nc.sync.dma_start(table_hi[HI_R - 1:HI_R, :], negbig[:])

                # ---- conv1 gather index loads ----
                gi1 = {}
                for si, nm in ((0, "gi1_lo"), (1, "gi1_hi")):
                    t = sing.tile([128, max(W1lo, W2lo) if si == 0
                                   else max(W1hi, W2hi)], i16, tag=f"gi_{si}")
                    nc.scalar.dma_start(t[:, 0:gi_in[nm].shape[1]],
                                        gi_in[nm][:])
                    gi1[si] = t

                # ---- conv1 A-table build (p-major batches) ----
                for c0 in range(0, NPAD1, CB1):
                    G = min(CB1 // 128, _ceil(N - c0, 128))
                    lhs = ld.tile([CIN + 3, CB1], f16, tag="lhs1", bufs=3)
                    nc.sync.dma_start(lhs[:], xpT_in[:, c0:c0 + CB1])
                    oc = ld.tile([128, CB1 // 128, COUT], f16, tag="oc", bufs=3)
                    for g2_ in range(_ceil(G, 2)):
                        pb = ppb.tile([128, 2, COUT], f32, tag="pb")
                        for h in range(min(2, G - g2_ * 2)):
                            g = g2_ * 2 + h
                            nc.tensor.matmul(
                                out=pb[:, h, :],
                                lhsT=lhs[:, g * 128:(g + 1) * 128],
                                rhs=W["W1s"][:], start=True, stop=True)
                        nh = min(2, G - g2_ * 2)
                        if g2_ & 1:
                            nc.scalar.copy(out=oc[:, g2_ * 2:g2_ * 2 + nh, :],
                                           in_=pb[:, 0:nh, :])
                        else:
                            nc.vector.tensor_copy(
                                out=oc[:, g2_ * 2:g2_ * 2 + nh, :],
                                in_=pb[:, 0:nh, :])
                    table_write_pm(oc, 1 + c0, G)

                # ---- conv1 gathers ----
                acc = big.tile([128, 2 * SLOC, COUT], f16, tag="acc")
                nc.vector.memset(acc[:], BIG_NEG)
                hl1 = gathers(gi1, sched1, acc, mg1)

                # ---- skip path: skipT = Wl^T @ xsT2 (lo2 order) ----
                skipT = big.tile([COUT, NSLOT], f16, tag="skipT")
                sks = [None, None]
                skq = [None, None]
                nsk = _ceil(NSLOT, VC)
                for ci in range(nsk):
                    j0 = ci * VC
                    hw = min(VC, NSLOT - j0)
                    xs = ld.tile([CIN, VC], f16, tag="lhs2")
                    nc.sync.dma_start(xs[:, 0:hw], xsT2_in[:, j0:j0 + hw])
                    pskip = pcs.tile([128, VC], f32, tag="cps")
                    nc.tensor.matmul(out=pskip[:, 0:hw], lhsT=W["Wls"][:],
                                     rhs=xs[:, 0:hw], start=True, stop=True)
                    nc.scalar.copy(out=skipT[:, j0:j0 + hw],
                                   in_=pskip[:, 0:hw])
                    ps = ck.tile([COUT, 1], f32, tag="ps")
                    nc.vector.tensor_reduce(out=ps[:],
                                            in_=skipT[:, j0:j0 + hw],
                                            op=OP.add, axis=AXX)
                    junk = ck.tile([128, VC], f16, tag="junk")
                    nc.vector.tensor_tensor(out=junk[:, 0:hw],
                                            in0=skipT[:, j0:j0 + hw],
                                            in1=skipT[:, j0:j0 + hw],
                                            op=OP.mult)
                    pq = ck.tile([COUT, 1], f32, tag="pq")
                    nc.vector.tensor_reduce(out=pq[:], in_=junk[:, 0:hw],
                                            op=OP.add, axis=AXX)
                    cs = ck.tile([COUT, 1], f32, tag=f"ss{ci & 1}")
                    cq = ck.tile([COUT, 1], f32, tag=f"sq{ci & 1}")
                    if ci == 0:
                        nc.vector.tensor_copy(out=cs[:], in_=ps[:])
                        nc.vector.tensor_copy(out=cq[:], in_=pq[:])
                    else:
                        nc.vector.tensor_tensor(out=cs[:], in0=ps[:],
                                                in1=sks[(ci - 1) & 1][:],
                                                op=OP.add)
                        nc.vector.tensor_tensor(out=cq[:], in0=pq[:],
                                                in1=skq[(ci - 1) & 1][:],
                                                op=OP.add)
                    sks[ci & 1] = cs
                    skq[ci & 1] = cq
                sksum, sksq = sks[(nsk - 1) & 1], skq[(nsk - 1) & 1]

                # ---- conv1 merge + v1 + stats ----
                posm1 = sing.tile([5, NSLOT], f16, tag="posm")
                nc.sync.dma_start(posm1[:], posm1_in[:])
                pwps = sing.tile([COUT, NSLOT], f16, tag="pwps")
                nc.scalar.dma_start(pwps[:], pwps_in[:])
                v1T = big.tile([COUT, NSLOT], f16, tag="vT")
                s1, q1 = merge_and_v(acc, hl1, posm1, W["W1pa"], v1T)

                # ---- conv2 gather index loads (sync idle here) ----
                gi2 = {}
                for si, nm in ((0, "gi2_lo"), (1, "gi2_hi")):
                    t = sing.tile([128, max(W1lo, W2lo) if si == 0
                                   else max(W1hi, W2hi)], i16, tag=f"gi_{si}")
                    nc.sync.dma_start(t[:, 0:gi_in[nm].shape[1]], gi_in[nm][:])
                    gi2[si] = t

                arst = sing.tile([COUT, 4], f32, tag="arst")
                nc.vector.tensor_copy(out=arst[:, 0:1], in_=s1[:])
                nc.vector.tensor_copy(out=arst[:, 1:2], in_=q1[:])
                nc.vector.tensor_copy(out=arst[:, 2:3], in_=sksum[:])
                nc.vector.tensor_copy(out=arst[:, 3:4], in_=sksq[:])
                nc.sync.dma_start(ar_i[:], arst[:])
                nc.gpsimd.collective_compute(
                    "AllReduce", OP.add, replica_groups=groups,
                    ins=[ar_i[:]], outs=[ar_o[:]])
                arres = sing.tile([COUT, 4], f32, tag="arres")
                nc.sync.dma_start(arres[:], ar_o[:])

                sc1, sh1 = bn_params(arres[:, 0:1], arres[:, 1:2],
                                     pvec["g1"], pvec["be1"], "bn1")
                scl, shl = bn_params(arres[:, 2:3], arres[:, 3:4],
                                     pvec["gl"], pvec["bel"], "bnl")

                # h' = relu(bn1(v1)) + pos@P, once per core (in place over
                # v1T), then AllGathered
                nc.scalar.activation(out=v1T[:], in_=v1T[:], func=AF.Relu,
                                     bias=sh1[:], scale=sc1[:])
                nc.vector.tensor_tensor(out=v1T[:], in0=v1T[:], in1=pwps[:],
                                        op=OP.add)
                if ag_ib is not None:
                    nc.sync.dma_start(ag_ia[:], v1T[:, 0:HAG])
                    nc.gpsimd.collective_compute(
                        "AllGather", OP.bypass, replica_groups=groups,
                        ins=[ag_ia[:]], outs=[ag_oa[:]])
                    nc.sync.dma_start(ag_ib[:], v1T[:, HAG:NSLOT])
                    nc.gpsimd.collective_compute(
                        "AllGather", OP.bypass, replica_groups=groups,
                        ins=[ag_ib[:]], outs=[ag_ob[:]])
                else:
                    nc.sync.dma_start(ag_ia[:], v1T[:])
                    nc.gpsimd.collective_compute(
                        "AllGather", OP.bypass, replica_groups=groups,
                        ins=[ag_ia[:]], outs=[ag_oa[:]])

                # ---- conv2 A-table build (p-major) ----
                for ct in range(NC):
                    for c0 in range(0, NSLOT, CB2):
                        w = min(CB2, NSLOT - c0)
                        nw = w // 128
                        lhs = ld.tile([COUT, CB2], f16, tag="vstg")
                        if c0 < HAG:
                            nc.sync.dma_start(lhs[:, 0:w],
                                              ag_oa[ct, :, c0:c0 + w])
                        else:
                            nc.sync.dma_start(
                                lhs[:, 0:w],
                                ag_ob[ct, :, c0 - HAG:c0 - HAG + w])
                        oc = ld.tile([128, CB2 // 128, COUT], f16, tag="oc2")
                        for g2_ in range(_ceil(nw, 2)):
                            pb = ppb.tile([128, 2, COUT], f32, tag="pb")
                            for h in range(min(2, nw - g2_ * 2)):
                                g = g2_ * 2 + h
                                nc.tensor.matmul(
                                    out=pb[:, h, :],
                                    lhsT=lhs[:, g * 128:(g + 1) * 128],
                                    rhs=W["W2as"][:], start=True, stop=True)
                            nh = min(2, nw - g2_ * 2)
                            if g2_ & 1:
                                nc.scalar.copy(
                                    out=oc[:, g2_ * 2:g2_ * 2 + nh, :],
                                    in_=pb[:, 0:nh, :])
                            else:
                                nc.vector.tensor_copy(
                                    out=oc[:, g2_ * 2:g2_ * 2 + nh, :],
                                    in_=pb[:, 0:nh, :])
                        table_write_pm(oc, 1 + ct * NSLOT + c0, nw)

                # ---- precompute bnl(skip) while conv2 gathers run ----
                bskT = big.tile([COUT, NSLOT], f16, tag="skip2")
                for j0 in range(0, NSLOT, VC):
                    w = min(VC, NSLOT - j0)
                    nc.scalar.activation(out=bskT[:, j0:j0 + w],
                                         in_=skipT[:, j0:j0 + w],
                                         func=AF.Identity,
                                         bias=shl[:], scale=scl[:])

                # ---- conv2 gathers ----
                acc2 = big.tile([128, 2 * SLOC, COUT], f16, tag="acc")
                nc.vector.memset(acc2[:], BIG_NEG)
                hl2 = gathers(gi2, sched2, acc2, mg2)

                # ---- conv2 merge + v2 + stats ----
                posm2 = sing.tile([5, NSLOT], f16, tag="posm")
                nc.sync.dma_start(posm2[:], posm2_in[:])
                v2T = big.tile([COUT, NSLOT], f16, tag="vT")
                s2, q2 = merge_and_v(acc2, hl2, posm2, W["W2pa"], v2T)

                arst2 = sing.tile([COUT, 2], f32, tag="arst2")
                nc.vector.tensor_copy(out=arst2[:, 0:1], in_=s2[:])
                nc.vector.tensor_copy(out=arst2[:, 1:2], in_=q2[:])
                nc.sync.dma_start(ar2_i[:], arst2[:])
                nc.gpsimd.collective_compute(
                    "AllReduce", OP.add, replica_groups=groups,
                    ins=[ar2_i[:]], outs=[ar2_o[:]])
                arres2 = sing.tile([COUT, 2], f32, tag="arres2")
                nc.sync.dma_start(arres2[:], ar2_o[:])
                sc2, sh2 = bn_params(arres2[:, 0:1], arres2[:, 1:2],
                                     pvec["g2"], pvec["be2"], "bn2")

                # ---- final = relu(bn2(v2) + bnl(skip)) ----
                for j0 in range(0, NSLOT, VC):
                    w = min(VC, NSLOT - j0)
                    a_ = ck.tile([128, VC], f32, tag="aggf")
                    nc.scalar.activation(out=a_[:, 0:w],
                                         in_=v2T[:, j0:j0 + w],
                                         func=AF.Identity,
                                         bias=sh2[:], scale=sc2[:])
                    fin = ck.tile([128, VC], f16, tag="fin")
                    nc.vector.tensor_tensor(out=fin[:, 0:w], in0=a_[:, 0:w],
                                            in1=bskT[:, j0:j0 + w], op=OP.add)
                    nc.vector.tensor_scalar(out=fin[:, 0:w], in0=fin[:, 0:w],
                                            scalar1=0.0, scalar2=None,
                                            op0=OP.max)
                    nc.sync.dma_start(out_t[:, j0:j0 + w], fin[:, 0:w])

    nc.compile()
    return nc


def make_in_maps(inputs, cfg, per_core, shared):
    f16 = np.float16
    CIN, COUT = cfg["CIN"], cfg["COUT"]
    W1 = np.asarray(inputs["W1"], np.float32)
    b1 = np.asarray(inputs["b1"], np.float32)
    W2 = np.asarray(inputs["W2"], np.float32)
    b2 = np.asarray(inputs["b2"], np.float32)
    ones = np.ones((1, COUT), np.float32)
    W1pa = np.concatenate([W1[CIN:CIN + 3, :], b1[None, :], ones], axis=0)
    W2pa = np.concatenate([W2[COUT:COUT + 3, :], b2[None, :], ones], axis=0)
    W2a = W2[0:COUT, :].astype(np.float64)
    W2p = W2[COUT:COUT + 3, :].astype(np.float64)
    P = np.linalg.solve(W2a.T, W2p.T).T          # P @ W2a == W2p
    poswP = (P.T @ shared["posw"].astype(np.float64)).astype(f16)
    NSLOT = poswP.shape[1] // len(per_core)
    base = dict(
        xpT=shared["xpT"],
        W1s=W1.astype(f16), W1pa=W1pa.astype(f16),
        W2as=W2[0:COUT, :].astype(f16), W2pa=W2pa.astype(f16),
        Wls=np.asarray(inputs["Wl"], np.float32).astype(f16),
        g1=np.asarray(inputs["g1"], np.float32).reshape(-1, 1),
        be1=np.asarray(inputs["be1"], np.float32).reshape(-1, 1),
        g2=np.asarray(inputs["g2"], np.float32).reshape(-1, 1),
        be2=np.asarray(inputs["be2"], np.float32).reshape(-1, 1),
        gl=np.asarray(inputs["gl"], np.float32).reshape(-1, 1),
        bel=np.asarray(inputs["bel"], np.float32).reshape(-1, 1),
    )
    in_maps = []
    for c, pc in enumerate(per_core):
        m = dict(base)
        for k in ("gi1_lo", "gi1_hi", "gi2_lo", "gi2_hi",
                  "mg_hi1", "mg_hi2", "posm1", "posm2", "xsT2"):
            m[k] = pc[k]
        m["pwp_self"] = np.ascontiguousarray(
            poswP[:, c * NSLOT:(c + 1) * NSLOT])
        in_maps.append(m)
    return in_maps


_CACHE = {}


def run(inputs, cfg, use_sim=False, trace=False):
    per_core, shared, scheds = host_prep(
        inputs["edge_index"], inputs["pos"], inputs["x"], cfg)
    key = (cfg["N"], tuple(scheds[0][0]), tuple(scheds[0][1]),
           tuple(scheds[1][0]), tuple(scheds[1][1]))
    if key not in _CACHE:
        _CACHE[key] = build_bass(cfg, scheds)
    nc = _CACHE[key]
    in_maps = make_in_maps(inputs, cfg, per_core, shared)
    NC = cfg["NC"]
    NLOC = cfg["N"] // NC
    if use_sim:
        from concourse.bass_interp import MultiCoreSim
        sim = MultiCoreSim(nc, num_cores=NC, require_finite=False,
                           require_nnan=False)
        for c in range(NC):
            for k, v in in_maps[c].items():
                sim.cores[c].tensor(k)[:] = v
        sim.simulate(check_with_hw=False)
        outs = [np.array(sim.cores[c].tensor("out")) for c in range(NC)]
        res = None
    else:
        from concourse.bass_utils import run_bass_kernel_spmd
        res = run_bass_kernel_spmd(nc, in_maps, core_ids=list(range(NC)),
                                   trace=trace)
        outs = [res.results[c]["out"] for c in range(NC)]
    full = np.empty((cfg["N"], cfg["COUT"]), np.float32)
    for c in range(NC):
        order2 = per_core[c]["order2_lo"]
        real = order2 < NLOC
        full[c * NLOC + order2[real]] = outs[c].T[real].astype(np.float32)
    return full, res


def kernel(**inputs):
    out, _ = run(inputs, FULL_CFG, use_sim=False)
    return out


# revision 29
# speedup vs baseline: 1.1669x; 1.0945x over previous
"""Trainium2 Bass kernel for nn_BlockConv (PointNet-style GNN block), 8 cores.

Algebraic core: msg_e = concat(x_src, pos_src-pos_dst) @ W + b
  = A[src] - C[dst], with A = concat(x,pos)@W (per-node table) and
  C = pos@W[-3:] - b (per-dst, constant within a segment).
  segment_max over dst = (gather+max of A rows) - C[dst]. Memory-bound.

Distribution: dst-sharded; per-core edge gathers from a replicated fp16
A-table in HBM via 4-queue SWDGE dma_gather (descriptor-rate limited:
~3ns/row on 4 queues vs ~9 on one). Tables, stages, h are fp16. Host
pre-transposes x/pos so table builds are straight fp16 matmuls; the
table uses a p-major row layout (row = base + p*G + g for batch column
p + 128*g) so each table write is one contiguous-per-partition DMA
instead of a 2048-descriptor strided store. Dst slots are degree-sorted
per side (lo/hi row windows for int16 idx) so gather pass k covers a
slot prefix. Dead (degree-0) slots get an exact v=0 via a 5th posm row
carrying -BIG through the C-matmul. The AllGather ships pre-BN v1^T in
two column chunks (first issued mid-merge) concurrently with the stats
AllReduce; conv2's build applies BN+relu on the fly (DVE) and folds
pos@W2p into h via host-computed P = W2p @ W2a^-1. Output is
feature-major in lo2-slot order; the host unpermutes.
"""
import sys
import numpy as np

if "/opt/trn_rl_repo" not in sys.path:
    sys.path.insert(0, "/opt/trn_rl_repo")

BIG_NEG = -60000.0
EPS = 1e-5
CB1 = 2048          # conv1 table build batch (columns)
CB2 = 2048          # conv2 table build batch (columns)

FULL_CFG = dict(N=50000, E=800000, CIN=64, COUT=128, NC=8,
                LO_LIM=32768, R=50432)
MINI_CFG = dict(N=2048, E=16384, CIN=64, COUT=128, NC=8,
                LO_LIM=1024, R=2432)
MID_CFG = dict(N=16384, E=262144, CIN=64, COUT=128, NC=8,
               LO_LIM=8192, R=16768)


def _ceil(a, b):
    return (a + b - 1) // b


def _wrap16(ids):
    """flat int list (len % 128 == 0) -> [128, len//16] int16 wrapped:
    unwrapped[j] = g[j%16, j//16], replicated over the 8 core groups."""
    a = np.asarray(ids, np.int64)
    assert a.size % 128 == 0 and a.min() >= 0 and a.max() < 32768
    g = a.reshape(a.size // 16, 16).T.astype(np.int16)   # [16, L/16]
    return np.tile(g, (8, 1))                            # [128, L/16]


def _row1(n, N):
    """p-major conv1 table row for node n (batches of CB1 columns)."""
    b = n // CB1
    o = n - b * CB1
    G = np.minimum(CB1 // 128, (N - b * CB1 + 127) // 128)
    return 1 + b * CB1 + (o % 128) * G + o // 128


def _row2(ct, slot, NSLOT):
    """p-major conv2 table row for (core ct, lo1-slot)."""
    c0 = (slot // CB2) * CB2
    w = np.minimum(CB2, NSLOT - c0)
    o = slot - c0
    return 1 + ct * NSLOT + c0 + (o % 128) * (w // 128) + o // 128


def _side_prep(rows, d_loc, NSLOT):
    """Degree-sort dst slots for one source window. rows = side-local table
    row ids per edge; d_loc = local dst node position per edge."""
    deg = np.bincount(d_loc, minlength=NSLOT)
    order = np.argsort(-deg, kind="stable")        # slot s -> node position
    slot_of = np.empty(NSLOT, np.int64)            # node position -> slot
    slot_of[order] = np.arange(NSLOT)
    isort = np.argsort(d_loc, kind="stable")
    starts = np.zeros(NSLOT + 1, np.int64)
    np.cumsum(deg, out=starts[1:])
    return {"deg": deg, "order": order, "slot_of": slot_of,
            "s_sorted": rows[isort], "starts": starts,
            "cnts": np.sort(deg)[::-1]}


def _build_sched(sides_by_core):
    """sched[si] = list of S_k (128-slot chunks) per pass, max across cores."""
    sched = []
    for si in range(2):
        Sk = []
        kmax = max(int(sd[si]["cnts"][0]) for sd in sides_by_core)
        for k in range(kmax):
            cnt = max(int((sd[si]["cnts"] > k).sum()) for sd in sides_by_core)
            if cnt == 0:
                break
            Sk.append(_ceil(cnt, 128))
        sched.append(Sk)
    return sched


def _build_gi(side, sched_side, dummy):
    blocks = []
    for k, S in enumerate(sched_side):
        L = S * 128
        ids = np.full(L, dummy, np.int64)
        nsl = int((side["cnts"] > k).sum())
        nodes = side["order"][:nsl]
        ids[:nsl] = side["s_sorted"][side["starts"][nodes] + k]
        blocks.append(_wrap16(ids))
    if not blocks:
        return np.zeros((128, 8), np.int16)
    return np.concatenate(blocks, axis=1)


def _posm_aug(pos, node, dead):
    """[5, NSLOT] fp16: rows 0-2 pos, row 3 = -1 (bias), row 4 = -BIG flag.
    Dead slots get rows 0-3 zeroed and row4=-BIG so C' = -BIG exactly and
    v = agg - C' = 0 exactly."""
    m = np.empty((5, node.size), np.float16)
    m[:3] = pos[node].T
    m[3] = -1.0
    m[4] = 0.0
    m[0:4, dead] = 0.0
    m[4, dead] = BIG_NEG
    return np.ascontiguousarray(m)


def host_prep(edge_index, pos, x, cfg):
    N, NC, LO_LIM, R = cfg["N"], cfg["NC"], cfg["LO_LIM"], cfg["R"]
    NLOC = N // NC
    SLOC = _ceil(NLOC, 128)
    NSLOT = SLOC * 128
    HID = R - 1 - LO_LIM          # hi-local dummy row (global row R-1)
    src = np.asarray(edge_index[0], np.int64)
    dst = np.asarray(edge_index[1], np.int64)
    core_of = dst // NLOC
    pos = np.asarray(pos, np.float32)
    x = np.asarray(x, np.float32)

    # ---- conv1: per-core lo/hi sides in p-major row space ----
    r1 = _row1(src, N)
    data1, slot1_lo_glob = [], np.empty(N, np.int64)
    for c in range(NC):
        m = core_of == c
        r = r1[m]
        d = dst[m] - c * NLOC
        sides = []
        for si, sel in ((0, r < LO_LIM), (1, r >= LO_LIM)):
            sides.append(_side_prep(r[sel] - (LO_LIM if si else 0),
                                    d[sel], NSLOT))
        data1.append(sides)
        slot1_lo_glob[c * NLOC:(c + 1) * NLOC] = sides[0]["slot_of"][:NLOC]
    sched1 = _build_sched(data1)

    # ---- conv2: rows in p-major (core, lo1-slot) space ----
    r2 = _row2(src // NLOC, slot1_lo_glob[src], NSLOT)
    data2 = []
    for c in range(NC):
        m = core_of == c
        r = r2[m]
        d = dst[m] - c * NLOC
        sides = []
        for si, sel in ((0, r < LO_LIM), (1, r >= LO_LIM)):
            sides.append(_side_prep(r[sel] - (LO_LIM if si else 0),
                                    d[sel], NSLOT))
        data2.append(sides)
    sched2 = _build_sched(data2)

    # ---- per-core tensors ----
    per_core = []
    for c in range(NC):
        s1, s2 = data1[c], data2[c]
        deg_tot = s1[0]["deg"] + s1[1]["deg"]      # per node position
        node1 = np.minimum(c * NLOC + s1[0]["order"], N - 1)
        node2 = np.minimum(c * NLOC + s2[0]["order"], N - 1)
        xs2 = np.zeros((64, NSLOT), np.float16)
        real2 = s2[0]["order"] < NLOC
        xs2[:, real2] = x[c * NLOC + s2[0]["order"][real2]].T
        per_core.append({
            "gi1_lo": _build_gi(s1[0], sched1[0], 0),
            "gi1_hi": _build_gi(s1[1], sched1[1], HID),
            "gi2_lo": _build_gi(s2[0], sched2[0], 0),
            "gi2_hi": _build_gi(s2[1], sched2[1], HID),
            "mg_hi1": _wrap16(s1[1]["slot_of"][s1[0]["order"]]),
            "mg_hi2": _wrap16(s2[1]["slot_of"][s2[0]["order"]]),
            "posm1": _posm_aug(pos, node1, deg_tot[s1[0]["order"]] == 0),
            "posm2": _posm_aug(pos, node2, deg_tot[s2[0]["order"]] == 0),
            "xsT2": np.ascontiguousarray(xs2),
            "order2_lo": s2[0]["order"],
        })

    # ---- shared tensors ----
    NPAD1 = _ceil(N, CB1) * CB1
    xpT = np.zeros((cfg["CIN"] + 3, NPAD1), np.float16)
    xpT[:cfg["CIN"], :N] = x.T
    xpT[cfg["CIN"]:, :N] = pos.T
    posw = np.empty((3, NC * NSLOT), np.float32)
    for c in range(NC):
        node1 = np.minimum(c * NLOC + data1[c][0]["order"], N - 1)
        posw[:, c * NSLOT:(c + 1) * NSLOT] = pos[node1].T
    shared = {"xpT": np.ascontiguousarray(xpT), "posw": posw}
    return per_core, shared, (sched1, sched2)


def build_bass(cfg, scheds, reps=1):
    import concourse.bass as bass
    import concourse.bacc as bacc
    import concourse.tile as tile
    from concourse import mybir
    from concourse.masks import make_identity
    import contextlib

    N, NC = cfg["N"], cfg["NC"]
    CIN, COUT = cfg["CIN"], cfg["COUT"]
    NLOC = N // NC
    SLOC = _ceil(NLOC, 128)
    NSLOT = SLOC * 128
    LO_LIM, R = cfg["LO_LIM"], cfg["R"]
    HI_R = R - LO_LIM
    NPAD1 = _ceil(N, CB1) * CB1
    sched1, sched2 = scheds
    f32, f16, i16 = mybir.dt.float32, mybir.dt.float16, mybir.dt.int16
    OP = mybir.AluOpType
    AF = mybir.ActivationFunctionType
    AXX = mybir.AxisListType.X
    VC = min(512, NSLOT)          # v-compute chunk (slots)

    nc = bacc.Bacc(num_devices=NC, name="blockconv4", num_swdge_queues=4)

    xpT_in = nc.dram_tensor("xpT", [CIN + 3, NPAD1], f16, kind="ExternalInput")
    xsT2_in = nc.dram_tensor("xsT2", [CIN, NSLOT], f16, kind="ExternalInput")
    posm1_in = nc.dram_tensor("posm1", [5, NSLOT], f16, kind="ExternalInput")
    posm2_in = nc.dram_tensor("posm2", [5, NSLOT], f16, kind="ExternalInput")
    pwps_in = nc.dram_tensor("pwp_self", [COUT, NSLOT], f16,
                             kind="ExternalInput")
    wt = {}
    for nm, shp, dt in (("W1s", [CIN + 3, COUT], f16), ("W1pa", [5, COUT], f16),
                        ("W2as", [COUT, COUT], f16), ("W2pa", [5, COUT], f16),
                        ("Wls", [CIN, COUT], f16),
                        ("g1", [COUT, 1], f32), ("be1", [COUT, 1], f32),
                        ("g2", [COUT, 1], f32), ("be2", [COUT, 1], f32),
                        ("gl", [COUT, 1], f32), ("bel", [COUT, 1], f32)):
        wt[nm] = nc.dram_tensor(nm, shp, dt, kind="ExternalInput")

    W1lo = max(sum(sched1[0]), 1) * 8
    W1hi = max(sum(sched1[1]), 1) * 8
    W2lo = max(sum(sched2[0]), 1) * 8
    W2hi = max(sum(sched2[1]), 1) * 8
    gi_in = {}
    for nm, w in (("gi1_lo", W1lo), ("gi1_hi", W1hi),
                  ("gi2_lo", W2lo), ("gi2_hi", W2hi)):
        gi_in[nm] = nc.dram_tensor(nm, [128, w], i16, kind="ExternalInput")
    mg1_in = nc.dram_tensor("mg_hi1", [128, NSLOT // 16], i16, kind="ExternalInput")
    mg2_in = nc.dram_tensor("mg_hi2", [128, NSLOT // 16], i16, kind="ExternalInput")

    out_t = nc.dram_tensor("out", [COUT, NSLOT], f16, kind="ExternalOutput")

    table_lo = nc.dram_tensor("table_lo", [LO_LIM, COUT], f16)
    table_hi = nc.dram_tensor("table_hi", [HI_R, COUT], f16)
    mbuf = nc.dram_tensor("mbuf", [NSLOT, COUT], f16)
    HAG = min(CB2, NSLOT)         # first AllGather chunk (build-aligned)
    ag_ia = nc.dram_tensor("ag_in_a", [COUT, HAG], f16)
    ag_oa = nc.dram_tensor("ag_out_a", [NC, COUT, HAG], f16,
                           addr_space="Shared")
    if NSLOT > HAG:
        ag_ib = nc.dram_tensor("ag_in_b", [COUT, NSLOT - HAG], f16)
        ag_ob = nc.dram_tensor("ag_out_b", [NC, COUT, NSLOT - HAG], f16,
                               addr_space="Shared")
    else:
        ag_ib = ag_ob = None
    ar_i = nc.dram_tensor("ar_in", [COUT, 4], f32)
    ar_o = nc.dram_tensor("ar_out", [COUT, 4], f32, addr_space="Shared")
    ar2_i = nc.dram_tensor("ar2_in", [COUT, 2], f32)
    ar2_o = nc.dram_tensor("ar2_out", [COUT, 2], f32, addr_space="Shared")
    bar_i = nc.dram_tensor("bar_in", [COUT, 1], f32)
    bar_o = nc.dram_tensor("bar_out", [COUT, 1], f32, addr_space="Shared")
    groups = [list(range(NC))]

    qctr = [0]

    def nextq():
        q = qctr[0] & 3
        qctr[0] += 1
        return q

    with tile.TileContext(nc) as tc:
        ctx = contextlib.ExitStack()
        with ctx:
            sing = ctx.enter_context(tc.tile_pool(name="sing", bufs=1))
            ld = ctx.enter_context(tc.tile_pool(name="ld", bufs=2))
            st = ctx.enter_context(tc.tile_pool(name="st", bufs=2))
            big = ctx.enter_context(tc.tile_pool(name="big", bufs=1))
            ck = ctx.enter_context(tc.tile_pool(name="ck", bufs=2))
            ppb = ctx.enter_context(tc.tile_pool(name="ppb", bufs=4, space="PSUM"))
            pcs = ctx.enter_context(tc.tile_pool(name="pcs", bufs=2, space="PSUM"))
            ppt = ctx.enter_context(tc.tile_pool(name="ppt", bufs=2, space="PSUM"))

            ident = sing.tile([128, 128], f16)
            make_identity(nc, ident)
            negbig = sing.tile([1, COUT], f16)
            nc.vector.memset(negbig[:], BIG_NEG)
            epsv = sing.tile([COUT, 1], f32)
            nc.vector.memset(epsv[:], EPS)

            W = {}
            for nm in ("W1s", "W1pa", "W2as", "W2pa", "Wls"):
                t = sing.tile(list(wt[nm].shape), f16, tag=f"w_{nm}")
                nc.sync.dma_start(t[:], wt[nm][:])
                W[nm] = t
            pvec = {}
            for nm in ("g1", "be1", "g2", "be2", "gl", "bel"):
                v = sing.tile([COUT, 1], f32, tag=f"pv_{nm}")
                nc.sync.dma_start(v[:], wt[nm][:])
                pvec[nm] = v

            mg1 = sing.tile([128, NSLOT // 16], i16, tag="mg1")
            nc.sync.dma_start(mg1[:], mg1_in[:])
            mg2 = sing.tile([128, NSLOT // 16], i16, tag="mg2")
            nc.sync.dma_start(mg2[:], mg2_in[:])

            def table_write_pm(oc, base, G):
                """p-major write: table row base + p*G + g <- oc[p, g, :].
                The lo/hi cut is partition-contiguous, so <=4 DMAs."""
                n = G * 128
                m0 = max(0, min(LO_LIM - base, n))
                pf, rem = divmod(m0, G)
                if pf:
                    d = table_lo[base:base + pf * G, :].rearrange(
                        "(p g) f -> p g f", g=G)
                    nc.scalar.dma_start(d, oc[0:pf, 0:G, :])
                if rem:
                    nc.scalar.dma_start(
                        table_lo[base + pf * G:base + m0, :].rearrange(
                            "(o r) f -> o r f", o=1),
                        oc[pf:pf + 1, 0:rem, :])
                if m0 < n:
                    b2 = base + m0 - LO_LIM
                    if rem:
                        nc.scalar.dma_start(
                            table_hi[b2:b2 + G - rem, :].rearrange(
                                "(o r) f -> o r f", o=1),
                            oc[pf:pf + 1, rem:G, :])
                        b2 += G - rem
                        pf += 1
                    if pf < 128:
                        d = table_hi[b2:b2 + (128 - pf) * G, :].rearrange(
                            "(p g) f -> p g f", g=G)
                        nc.scalar.dma_start(d, oc[pf:128, 0:G, :])

            def side_passes(idxt, sched_side, win, winsz, acc, ro):
                off = 0
                for k, S in enumerate(sched_side):
                    stg = st.tile([128, SLOC, COUT], f16, tag="stage", bufs=3)
                    a = 0
                    while a < S:
                        b = min(a + 8, S)
                        nc.gpsimd.dma_gather(
                            out_ap=stg[:, a:b, :], in_ap=win[0:winsz, :],
                            idxs_ap=idxt[:, off + a * 8:off + b * 8],
                            num_idxs=(b - a) * 128,
                            num_idxs_reg=(b - a) * 128,
                            elem_size=COUT, queue_num=nextq())
                        a = b
                    nc.vector.tensor_tensor(
                        out=acc[:, ro:ro + S, :], in0=acc[:, ro:ro + S, :],
                        in1=stg[:, 0:S, :], op=OP.max)
                    off += S * 8

            def gathers(gi, sched_c, acc, mg):
                """lo then hi passes; mbuf round-trip regather of the hi
                half into lo-slot order."""
                side_passes(gi[0], sched_c[0], table_lo, LO_LIM, acc, 0)
                side_passes(gi[1], sched_c[1], table_hi, HI_R, acc, SLOC)
                nc.sync.dma_start(
                    mbuf[:].rearrange("(s p) f -> p s f", p=128),
                    acc[:, SLOC:2 * SLOC, :])
                hi_lo = st.tile([128, SLOC, COUT], f16, tag="stage", bufs=3)
                a = 0
                while a < SLOC:
                    b = min(a + 8, SLOC)
                    nc.gpsimd.dma_gather(
                        out_ap=hi_lo[:, a:b, :], in_ap=mbuf[:, :],
                        idxs_ap=mg[:, a * 8:b * 8],
                        num_idxs=(b - a) * 128, num_idxs_reg=(b - a) * 128,
                        elem_size=COUT, queue_num=nextq())
                    a = b
                return hi_lo

            def merge_and_v(acc, hi_lo, posm_t, Wp_aug, vT):
                """agg = max(acc lo, hi_lo regather) -> transpose ->
                v^T = agg^T - C'."""
                nc.vector.tensor_tensor(out=hi_lo[:], in0=hi_lo[:],
                                        in1=acc[:, 0:SLOC, :], op=OP.max)
                ssum = [None, None]
                sqq = [None, None]
                nchunk = _ceil(NSLOT, VC)
                for ci in range(nchunk):
                    j0 = ci * VC
                    nr = min(VC, NSLOT - j0) // 128
                    w = nr * 128
                    pt = ppt.tile([128, VC // 128, 128], f16, tag="pt")
                    for r in range(nr):
                        nc.tensor.transpose(
                            out=pt[:, r, :],
                            in_=hi_lo[:, j0 // 128 + r, :],
                            identity=ident[:])
                    ptf = pt[:, 0:nr, :].rearrange("p a b -> p (a b)")
                    aggf = ck.tile([128, VC], f32, tag="aggf")
                    nc.scalar.copy(out=aggf[:, 0:w], in_=ptf)
                    cps = pcs.tile([128, VC], f32, tag="cps")
                    nc.tensor.matmul(out=cps[:, 0:w], lhsT=Wp_aug[:],
                                     rhs=posm_t[:, j0:j0 + w],
                                     start=True, stop=True)
                    nc.vector.tensor_tensor(out=vT[:, j0:j0 + w],
                                            in0=aggf[:, 0:w], in1=cps[:, 0:w],
                                            op=OP.subtract)
                    ps = ck.tile([COUT, 1], f32, tag="ps")
                    nc.vector.tensor_reduce(out=ps[:], in_=vT[:, j0:j0 + w],
                                            op=OP.add, axis=AXX)
                    junk = ck.tile([128, VC], f16, tag="junk")
                    nc.vector.tensor_tensor(out=junk[:, 0:w],
                                            in0=vT[:, j0:j0 + w],
                                            in1=vT[:, j0:j0 + w], op=OP.mult)
                    pq = ck.tile([COUT, 1], f32, tag="pq")
                    nc.vector.tensor_reduce(out=pq[:], in_=junk[:, 0:w],
                                            op=OP.add, axis=AXX)
                    cs = ck.tile([COUT, 1], f32, tag=f"ms{ci & 1}")
                    cq = ck.tile([COUT, 1], f32, tag=f"mq{ci & 1}")
                    if ci == 0:
                        nc.vector.tensor_copy(out=cs[:], in_=ps[:])
                        nc.vector.tensor_copy(out=cq[:], in_=pq[:])
                    else:
                        nc.vector.tensor_tensor(out=cs[:], in0=ps[:],
                                                in1=ssum[(ci - 1) & 1][:],
                                                op=OP.add)
                        nc.vector.tensor_tensor(out=cq[:], in0=pq[:],
                                                in1=sqq[(ci - 1) & 1][:],
                                                op=OP.add)
                    ssum[ci & 1] = cs
                    sqq[ci & 1] = cq
                return ssum[(nchunk - 1) & 1], sqq[(nchunk - 1) & 1]

            def bn_params(sum_ap, sq_ap, g_v, be_v, tagp):
                """scale = g*rsqrt(var+eps), shift = be - mean*scale; [COUT,1]."""
                mean = ck.tile([COUT, 1], f32, tag=f"{tagp}_m")
                nc.vector.tensor_scalar(out=mean[:], in0=sum_ap, scalar1=1.0 / N,
                                        scalar2=None, op0=OP.mult)
                ex2 = ck.tile([COUT, 1], f32, tag=f"{tagp}_e")
                nc.vector.tensor_scalar(out=ex2[:], in0=sq_ap, scalar1=1.0 / N,
                                        scalar2=None, op0=OP.mult)
                m2 = ck.tile([COUT, 1], f32, tag=f"{tagp}_m2")
                nc.vector.tensor_tensor(out=m2[:], in0=mean[:], in1=mean[:],
                                        op=OP.mult)
                var = ck.tile([COUT, 1], f32, tag=f"{tagp}_v")
                nc.vector.tensor_tensor(out=var[:], in0=ex2[:], in1=m2[:],
                                        op=OP.subtract)
                sd = ck.tile([COUT, 1], f32, tag=f"{tagp}_sd")
                nc.scalar.activation(out=sd[:], in_=var[:], func=AF.Sqrt,
                                     bias=epsv[:], scale=1.0)
                rstd = ck.tile([COUT, 1], f32, tag=f"{tagp}_r")
                nc.vector.reciprocal(out=rstd[:], in_=sd[:])
                sc = sing.tile([COUT, 1], f32, tag=f"{tagp}_sc")
                nc.vector.tensor_tensor(out=sc[:], in0=rstd[:], in1=g_v[:],
                                        op=OP.mult)
                ms = ck.tile([COUT, 1], f32, tag=f"{tagp}_ms")
                nc.vector.tensor_tensor(out=ms[:], in0=mean[:], in1=sc[:],
                                        op=OP.mult)
                sh = sing.tile([COUT, 1], f32, tag=f"{tagp}_sh")
                nc.vector.tensor_tensor(out=sh[:], in0=be_v[:], in1=ms[:],
                                        op=OP.subtract)
                return sc, sh

            for _rep in range(reps):
                if _rep == 0:
                    # pre-barrier: overlap cross-core rendezvous with build
                    nc.sync.dma_start(bar_i[:], epsv[:])
                    nc.gpsimd.collective_compute(
                        "AllReduce", OP.add, replica_groups=groups,
                        ins=[bar_i[:]], outs=[bar_o[:]])

                # ---- dummy rows ----
                nc.sync.dma_start(table_lo[0:1, :], negbig[:])
                nc.sync.dma_start(table_hi[HI_R - 1:HI_R, :], negbig[:])

                # ---- conv1 gather index loads ----
                gi1 = {}
                for si, nm in ((0, "gi1_lo"), (1, "gi1_hi")):
                    t = sing.tile([128, max(W1lo, W2lo) if si == 0
                                   else max(W1hi, W2hi)], i16, tag=f"gi_{si}")
                    nc.scalar.dma_start(t[:, 0:gi_in[nm].shape[1]],
                                        gi_in[nm][:])
                    gi1[si] = t

                # ---- conv1 A-table build (p-major batches) ----
                for c0 in range(0, NPAD1, CB1):
                    G = min(CB1 // 128, _ceil(N - c0, 128))
                    lhs = ld.tile([CIN + 3, CB1], f16, tag="lhs1", bufs=4)
                    hrows = (CIN + 3) // 2
                    nc.sync.dma_start(lhs[0:hrows, :],
                                      xpT_in[0:hrows, c0:c0 + CB1])
                    nc.scalar.dma_start(lhs[hrows:CIN + 3, :],
                                        xpT_in[hrows:CIN + 3, c0:c0 + CB1])
                    oc = ld.tile([128, CB1 // 128, COUT], f16, tag="oc", bufs=4)
                    for g2_ in range(_ceil(G, 2)):
                        pb = ppb.tile([128, 2, COUT], f32, tag="pb")
                        for h in range(min(2, G - g2_ * 2)):
                            g = g2_ * 2 + h
                            nc.tensor.matmul(
                                out=pb[:, h, :],
                                lhsT=lhs[:, g * 128:(g + 1) * 128],
                                rhs=W["W1s"][:], start=True, stop=True)
                        nh = min(2, G - g2_ * 2)
                        if g2_ & 1:
                            nc.scalar.copy(out=oc[:, g2_ * 2:g2_ * 2 + nh, :],
                                           in_=pb[:, 0:nh, :])
                        else:
                            nc.vector.tensor_copy(
                                out=oc[:, g2_ * 2:g2_ * 2 + nh, :],
                                in_=pb[:, 0:nh, :])
                    table_write_pm(oc, 1 + c0, G)

                # ---- conv1 gathers ----
                acc = big.tile([128, 2 * SLOC, COUT], f16, tag="acc")
                nc.vector.memset(acc[:], BIG_NEG)
                hl1 = gathers(gi1, sched1, acc, mg1)

                # ---- skip path: skipT = Wl^T @ xsT2 (lo2 order) ----
                skipT = big.tile([COUT, NSLOT], f16, tag="skipT")
                sks = [None, None]
                skq = [None, None]
                nsk = _ceil(NSLOT, VC)
                for ci in range(nsk):
                    j0 = ci * VC
                    hw = min(VC, NSLOT - j0)
                    xs = ld.tile([CIN, VC], f16, tag="lhs2")
                    nc.sync.dma_start(xs[:, 0:hw], xsT2_in[:, j0:j0 + hw])
                    pskip = pcs.tile([128, VC], f32, tag="cps")
                    nc.tensor.matmul(out=pskip[:, 0:hw], lhsT=W["Wls"][:],
                                     rhs=xs[:, 0:hw], start=True, stop=True)
                    nc.scalar.copy(out=skipT[:, j0:j0 + hw],
                                   in_=pskip[:, 0:hw])
                    ps = ck.tile([COUT, 1], f32, tag="ps")
                    nc.vector.tensor_reduce(out=ps[:],
                                            in_=skipT[:, j0:j0 + hw],
                                            op=OP.add, axis=AXX)
                    junk = ck.tile([128, VC], f16, tag="junk")
                    nc.vector.tensor_tensor(out=junk[:, 0:hw],
                                            in0=skipT[:, j0:j0 + hw],
                                            in1=skipT[:, j0:j0 + hw],
                                            op=OP.mult)
                    pq = ck.tile([COUT, 1], f32, tag="pq")
                    nc.vector.tensor_reduce(out=pq[:], in_=junk[:, 0:hw],
                                            op=OP.add, axis=AXX)
                    cs = ck.tile([COUT, 1], f32, tag=f"ss{ci & 1}")
                    cq = ck.tile([COUT, 1], f32, tag=f"sq{ci & 1}")
                    if ci == 0:
                        nc.vector.tensor_copy(out=cs[:], in_=ps[:])
                        nc.vector.tensor_copy(out=cq[:], in_=pq[:])
                    else:
                        nc.vector.tensor_tensor(out=cs[:], in0=ps[:],
                                                in1=sks[(ci - 1) & 1][:],
                                                op=OP.add)
                        nc.vector.tensor_tensor(out=cq[:], in0=pq[:],
                                                in1=skq[(ci - 1) & 1][:],
                                                op=OP.add)
                    sks[ci & 1] = cs
                    skq[ci & 1] = cq
                sksum, sksq = sks[(nsk - 1) & 1], skq[(nsk - 1) & 1]

                # ---- conv1 merge + v1 + stats ----
                posm1 = sing.tile([5, NSLOT], f16, tag="posm")
                nc.sync.dma_start(posm1[:], posm1_in[:])
                pwps = sing.tile([COUT, NSLOT], f16, tag="pwps")
                nc.scalar.dma_start(pwps[:], pwps_in[:])
                v1T = big.tile([COUT, NSLOT], f16, tag="vT")
                s1, q1 = merge_and_v(acc, hl1, posm1, W["W1pa"], v1T)

                # ---- conv2 gather index loads (sync idle here) ----
                gi2 = {}
                for si, nm in ((0, "gi2_lo"), (1, "gi2_hi")):
                    t = sing.tile([128, max(W1lo, W2lo) if si == 0
                                   else max(W1hi, W2hi)], i16, tag=f"gi_{si}")
                    nc.sync.dma_start(t[:, 0:gi_in[nm].shape[1]], gi_in[nm][:])
                    gi2[si] = t

                arst = sing.tile([COUT, 4], f32, tag="arst")
                nc.vector.tensor_copy(out=arst[:, 0:1], in_=s1[:])
                nc.vector.tensor_copy(out=arst[:, 1:2], in_=q1[:])
                nc.vector.tensor_copy(out=arst[:, 2:3], in_=sksum[:])
                nc.vector.tensor_copy(out=arst[:, 3:4], in_=sksq[:])
                nc.sync.dma_start(ar_i[:], arst[:])
                nc.gpsimd.collective_compute(
                    "AllReduce", OP.add, replica_groups=groups,
                    ins=[ar_i[:]], outs=[ar_o[:]])
                arres = sing.tile([COUT, 4], f32, tag="arres")
                nc.sync.dma_start(arres[:], ar_o[:])

                sc1, sh1 = bn_params(arres[:, 0:1], arres[:, 1:2],
                                     pvec["g1"], pvec["be1"], "bn1")
                scl, shl = bn_params(arres[:, 2:3], arres[:, 3:4],
                                     pvec["gl"], pvec["bel"], "bnl")

                # h' = relu(bn1(v1)) + pos@P, once per core (in place over
                # v1T), then AllGathered
                nc.scalar.activation(out=v1T[:], in_=v1T[:], func=AF.Relu,
                                     bias=sh1[:], scale=sc1[:])
                nc.vector.tensor_tensor(out=v1T[:], in0=v1T[:], in1=pwps[:],
                                        op=OP.add)
                if ag_ib is not None:
                    nc.sync.dma_start(ag_ia[:], v1T[:, 0:HAG])
                    nc.gpsimd.collective_compute(
                        "AllGather", OP.bypass, replica_groups=groups,
                        ins=[ag_ia[:]], outs=[ag_oa[:]])
                    nc.sync.dma_start(ag_ib[:], v1T[:, HAG:NSLOT])
                    nc.gpsimd.collective_compute(
                        "AllGather", OP.bypass, replica_groups=groups,
                        ins=[ag_ib[:]], outs=[ag_ob[:]])
                else:
                    nc.sync.dma_start(ag_ia[:], v1T[:])
                    nc.gpsimd.collective_compute(
                        "AllGather", OP.bypass, replica_groups=groups,
                        ins=[ag_ia[:]], outs=[ag_oa[:]])

                # ---- conv2 A-table build (p-major); AG-a-covered
                # column group first so AG-b hides under it ----
                c0_list = list(range(0, NSLOT, CB2))
                for c0 in c0_list:
                    for ct in range(NC):
                        w = min(CB2, NSLOT - c0)
                        nw = w // 128
                        lhs = ld.tile([COUT, CB2], f16, tag="vstg")
                        if c0 < HAG:
                            nc.sync.dma_start(lhs[:, 0:w],
                                              ag_oa[ct, :, c0:c0 + w])
                        else:
                            nc.sync.dma_start(
                                lhs[:, 0:w],
                                ag_ob[ct, :, c0 - HAG:c0 - HAG + w])
                        oc = ld.tile([128, CB2 // 128, COUT], f16, tag="oc2")
                        for g2_ in range(_ceil(nw, 2)):
                            pb = ppb.tile([128, 2, COUT], f32, tag="pb")
                            for h in range(min(2, nw - g2_ * 2)):
                                g = g2_ * 2 + h
                                nc.tensor.matmul(
                                    out=pb[:, h, :],
                                    lhsT=lhs[:, g * 128:(g + 1) * 128],
                                    rhs=W["W2as"][:], start=True, stop=True)
                            nh = min(2, nw - g2_ * 2)
                            if g2_ & 1:
                                nc.scalar.copy(
                                    out=oc[:, g2_ * 2:g2_ * 2 + nh, :],
                                    in_=pb[:, 0:nh, :])
                            else:
                                nc.vector.tensor_copy(
                                    out=oc[:, g2_ * 2:g2_ * 2 + nh, :],
                                    in_=pb[:, 0:nh, :])
                        table_write_pm(oc, 1 + ct * NSLOT + c0, nw)

                # ---- precompute bnl(skip) while conv2 gathers run ----
                bskT = big.tile([COUT, NSLOT], f16, tag="skip2")
                for j0 in range(0, NSLOT, VC):
                    w = min(VC, NSLOT - j0)
                    nc.scalar.activation(out=bskT[:, j0:j0 + w],
                                         in_=skipT[:, j0:j0 + w],
                                         func=AF.Identity,
                                         bias=shl[:], scale=scl[:])

                # ---- conv2 gathers ----
                acc2 = big.tile([128, 2 * SLOC, COUT], f16, tag="acc")
                nc.vector.memset(acc2[:], BIG_NEG)
                hl2 = gathers(gi2, sched2, acc2, mg2)

                # ---- conv2 merge + v2 + stats ----
                posm2 = sing.tile([5, NSLOT], f16, tag="posm")
                nc.sync.dma_start(posm2[:], posm2_in[:])
                v2T = big.tile([COUT, NSLOT], f16, tag="vT")
                s2, q2 = merge_and_v(acc2, hl2, posm2, W["W2pa"], v2T)

                arst2 = sing.tile([COUT, 2], f32, tag="arst2")
                nc.vector.tensor_copy(out=arst2[:, 0:1], in_=s2[:])
                nc.vector.tensor_copy(out=arst2[:, 1:2], in_=q2[:])
                nc.sync.dma_start(ar2_i[:], arst2[:])
                nc.gpsimd.collective_compute(
                    "AllReduce", OP.add, replica_groups=groups,
                    ins=[ar2_i[:]], outs=[ar2_o[:]])
                arres2 = sing.tile([COUT, 2], f32, tag="arres2")
                nc.sync.dma_start(arres2[:], ar2_o[:])
                sc2, sh2 = bn_params(arres2[:, 0:1], arres2[:, 1:2],
                                     pvec["g2"], pvec["be2"], "bn2")

                # ---- final = relu(bn2(v2) + bnl(skip)) ----
                for j0 in range(0, NSLOT, VC):
                    w = min(VC, NSLOT - j0)
                    a_ = ck.tile([128, VC], f32, tag="aggf")
                    nc.scalar.activation(out=a_[:, 0:w],
                                         in_=v2T[:, j0:j0 + w],
                                         func=AF.Identity,
                                         bias=sh2[:], scale=sc2[:])
                    fin = ck.tile([128, VC], f16, tag="fin")
                    nc.vector.tensor_tensor(out=fin[:, 0:w], in0=a_[:, 0:w],
                                            in1=bskT[:, j0:j0 + w], op=OP.add)
                    nc.vector.tensor_scalar(out=fin[:, 0:w], in0=fin[:, 0:w],
                                            scalar1=0.0, scalar2=None,
                                            op0=OP.max)
                    nc.sync.dma_start(out_t[:, j0:j0 + w], fin[:, 0:w])

    nc.compile()
    return nc


def make_in_maps(inputs, cfg, per_core, shared):
    f16 = np.float16
    CIN, COUT = cfg["CIN"], cfg["COUT"]
    W1 = np.asarray(inputs["W1"], np.float32)
    b1 = np.asarray(inputs["b1"], np.float32)
    W2 = np.asarray(inputs["W2"], np.float32)
    b2 = np.asarray(inputs["b2"], np.float32)
    ones = np.ones((1, COUT), np.float32)
    W1pa = np.concatenate([W1[CIN:CIN + 3, :], b1[None, :], ones], axis=0)
    W2pa = np.concatenate([W2[COUT:COUT + 3, :], b2[None, :], ones], axis=0)
    W2a = W2[0:COUT, :].astype(np.float64)
    W2p = W2[COUT:COUT + 3, :].astype(np.float64)
    P = np.linalg.solve(W2a.T, W2p.T).T          # P @ W2a == W2p
    poswP = (P.T @ shared["posw"].astype(np.float64)).astype(f16)
    NSLOT = poswP.shape[1] // len(per_core)
    base = dict(
        xpT=shared["xpT"],
        W1s=W1.astype(f16), W1pa=W1pa.astype(f16),
        W2as=W2[0:COUT, :].astype(f16), W2pa=W2pa.astype(f16),
        Wls=np.asarray(inputs["Wl"], np.float32).astype(f16),
        g1=np.asarray(inputs["g1"], np.float32).reshape(-1, 1),
        be1=np.asarray(inputs["be1"], np.float32).reshape(-1, 1),
        g2=np.asarray(inputs["g2"], np.float32).reshape(-1, 1),
        be2=np.asarray(inputs["be2"], np.float32).reshape(-1, 1),
        gl=np.asarray(inputs["gl"], np.float32).reshape(-1, 1),
        bel=np.asarray(inputs["bel"], np.float32).reshape(-1, 1),
    )
    in_maps = []
    for c, pc in enumerate(per_core):
        m = dict(base)
        for k in ("gi1_lo", "gi1_hi", "gi2_lo", "gi2_hi",
                  "mg_hi1", "mg_hi2", "posm1", "posm2", "xsT2"):
            m[k] = pc[k]
        m["pwp_self"] = np.ascontiguousarray(
            poswP[:, c * NSLOT:(c + 1) * NSLOT])
        in_maps.append(m)
    return in_maps


_CACHE = {}


def run(inputs, cfg, use_sim=False, trace=False):
    per_core, shared, scheds = host_prep(
        inputs["edge_index"], inputs["pos"], inputs["x"], cfg)
    key = (cfg["N"], tuple(scheds[0][0]), tuple(scheds[0][1]),
           tuple(scheds[1][0]), tuple(scheds[1][1]))
    if key not in _CACHE:
        _CACHE[key] = build_bass(cfg, scheds)
    nc = _CACHE[key]
    in_maps = make_in_maps(inputs, cfg, per_core, shared)
    NC = cfg["NC"]
    NLOC = cfg["N"] // NC
    if use_sim:
        from concourse.bass_interp import MultiCoreSim
        sim = MultiCoreSim(nc, num_cores=NC, require_finite=False,
                           require_nnan=False)
        for c in range(NC):
            for k, v in in_maps[c].items():
                sim.cores[c].tensor(k)[:] = v
        sim.simulate(check_with_hw=False)
        outs = [np.array(sim.cores[c].tensor("out")) for c in range(NC)]
        res = None
    else:
        from concourse.bass_utils import run_bass_kernel_spmd
        res = run_bass_kernel_spmd(nc, in_maps, core_ids=list(range(NC)),
                                   trace=trace)
        outs = [res.results[c]["out"] for c in range(NC)]
    full = np.empty((cfg["N"], cfg["COUT"]), np.float32)
    for c in range(NC):
        order2 = per_core[c]["order2_lo"]
        real = order2 < NLOC
        full[c * NLOC + order2[real]] = outs[c].T[real].astype(np.float32)
    return full, res


def kernel(**inputs):
    out, _ = run(inputs, FULL_CFG, use_sim=False)
    return out
